# revision 2
# baseline (speedup 1.0000x reference)
"""Trainium2 Bass kernel for CAttentionBlock — v6: PE-offloaded reductions.

Layout per tile of 128 windows (all 4 attentions):
  qb   [128w, (xi,tok,c)=4096] bf16, natural channel order c=h*32+d
  QT   (PE transpose) [c128, (half,t,w)] bf16 psum, per xi
  prod (DVE) [c128, (t,s,w)=2048] bf16 sbuf, per (xi,half)
  S    (PE mm: lhsT=prod chunk, rhs=ones4 blockdiag) [128w, (xi,t,s,h8)=512] f32 psum
  e/pn (ACT exp + DVE softmax) [128w, 512] bf16
  pnT  (PE transpose w/ stride-0 d4 replication) [(s,h,d4)128, (t,w)] bf16 psum, per xi
  kbT  (PE transpose) [(tok,h,d4)128, (d8,w)] bf16 psum -> sbuf copy (Pool), per xi
  pvT  (DVE) [(s,h,d4)128, (t,d8,w)=4096] bf16 sbuf, per xi
  avout(PE mm pairs: s-reduce via bd4 + residual via identity slice)
       [128w, (t,c)=1024] f32 psum, per xi
  xres (ACT copy) -> xsq [128w, (xi,t,c)] bf16 sbuf; sq (ACT Square)
  stats (DVE halving tree) -> mu/rs (ACT+DVE tiny)
  norm: 16x tensor_scalar (x-mu)*rs (4x mode), *w (DVE TT), +b -> f32 (Pool)
"""

import sys

for _p in ("/opt/trn_rl_repo",):
    if _p not in sys.path:
        sys.path.insert(0, _p)

import numpy as np

import bass_rust
import concourse.bass as bass
import concourse.tile as tile
from concourse import mybir
from concourse.bass_utils import run_bass_kernel_spmd
from concourse.masks import make_identity

F32 = mybir.dt.float32
BF16 = mybir.dt.bfloat16
ALU = mybir.AluOpType
ACT = mybir.ActivationFunctionType
AX = mybir.AxisListType

B, H, W, C = 8, 64, 64, 256
WS = 2
NH = 8
D = C // NH            # 32
NTOK = WS * WS         # 4
NW = (H // WS) * (W // WS)
NWI = H // WS          # 32
P = 128
NTILES = NW // P       # 8
NX = 4
EPS = 1e-5
INV_SQRT_D = 1.0 / float(np.sqrt(D))
XK = [1, 2, 3, 1]      # K/V input index per attention

FQ = NX * NTOK * C         # 4096
DEBUG_DUMP = None  # "e" | "pn" | "xres" | "stats" | "y1"
AV_RESID = True
AV_ATT = True


def _ap(ref, offset_delta, dims):
    return bass_rust.AP(ref.tensor, ref.offset + offset_delta, [list(d) for d in dims])


def build_kernel(reps=1):
    nc = bass.Bass("TRN2", target_bir_lowering=False, debug=False)

    ins = {
        name: nc.dram_tensor(name, [H, W, C], F32, kind="ExternalInput")
        for name in ("r", "g", "b", "ir")
    }
    ln_params = []
    for a in range(4):
        wv = nc.dram_tensor(f"ln{a + 1}_w", [C], F32, kind="ExternalInput")
        bv = nc.dram_tensor(f"ln{a + 1}_b", [C], F32, kind="ExternalInput")
        ln_params.append((wv, bv))
    out = nc.dram_tensor("out", [H, W, 4 * C], F32, kind="ExternalOutput")

    in_aps = [ins[n].ap() for n in ("r", "g", "b", "ir")]
    out_ap = out.ap()
    NT = NTILES * reps

    with tile.TileContext(nc) as tc:
        with (
            tc.tile_pool(name="const", bufs=1) as pconst,
            tc.tile_pool(name="pin", bufs=2) as pin,
            tc.tile_pool(name="pqb", bufs=2) as pqb,
            tc.tile_pool(name="pprod", bufs=3) as pprod,
            tc.tile_pool(name="pqts", bufs=2) as pqts,
            tc.tile_pool(name="ppvt", bufs=2) as ppvt,
            tc.tile_pool(name="pkbs", bufs=1) as pkbs,
            tc.tile_pool(name="pxsq", bufs=2) as pxsq,
            tc.tile_pool(name="pxtr", bufs=1) as pxtr,
            tc.tile_pool(name="py", bufs=2) as py,
            tc.tile_pool(name="psmall", bufs=2) as psmall,
            tc.tile_pool(name="zqt", bufs=1, space="PSUM") as zqt,
            tc.tile_pool(name="zpnt", bufs=1, space="PSUM") as zpnt,
            tc.tile_pool(name="zkbt", bufs=1, space="PSUM") as zkbt,
            tc.tile_pool(name="zs", bufs=1, space="PSUM") as zs,
            tc.tile_pool(name="zav", bufs=1, space="PSUM") as zav,
        ):
            # ---------------- constants ----------------
            wcat = pconst.tile([P, NX * C], F32, tag="wcat")
            bcat = pconst.tile([P, NX * C], F32, tag="bcat")
            epst = pconst.tile([P, 1], F32, tag="epst")
            nc.vector.memset(epst[:], EPS)
            for a, (wv, bv) in enumerate(ln_params):
                nc.sync.dma_start(
                    out=wcat[:, a * C : (a + 1) * C],
                    in_=_ap(wv.ap(), 0, [[0, P], [1, C]]),
                )
                nc.sync.dma_start(
                    out=bcat[:, a * C : (a + 1) * C],
                    in_=_ap(bv.ap(), 0, [[0, P], [1, C]]),
                )
            wb = pconst.tile([P, NX * C], BF16, tag="wb")
            nc.scalar.activation(out=wb[:], in_=wcat[:], func=ACT.Copy,
                                 bias=0.0, scale=1.0)

            ident = pconst.tile([P, P], BF16, tag="ident")
            make_identity(nc, ident[:])

            # ones4: [ (h4,d):128, 4 ]  1 at row p, col p//32
            ones4 = pconst.tile([P, 4], BF16, tag="ones4")
            nc.gpsimd.memset(ones4[:], 0.0)
            for h4 in range(4):
                nc.gpsimd.memset(ones4[32 * h4 : 32 * (h4 + 1), h4 : h4 + 1], 1.0)

            # bd4: [ (s,j):128, 32 ]  1 at col j = p mod 32 (identity32 per s block)
            bd4 = pconst.tile([P, 32], BF16, tag="bd4")
            nc.gpsimd.memset(bd4[:], 0.0)
            for s in range(4):
                make_identity(nc, bd4[32 * s : 32 * (s + 1), :], nomemset=True)

            # ---------------- per tile ----------------
            state = {}

            def phase_load(i):
                t = i % NTILES
                qcat = pin.tile([P, FQ], F32, tag="qcat")
                qr = qcat[:]
                for xi in range(4):
                    for qh in range(2):
                        src = _ap(
                            in_aps[xi],
                            (8 * t + qh) * W * C,
                            [[2 * W * C, 4], [2 * C, NWI], [1, 2 * C]],
                        )
                        dst = _ap(
                            qr, xi * NTOK * C + qh * 2 * C, [qr.ap[0], [1, 2 * C]]
                        )
                        nc.sync.dma_start(out=dst, in_=src)
                qb = pqb.tile([P, FQ], BF16, tag="qb")
                nc.scalar.activation(out=qb[:], in_=qcat[:], func=ACT.Copy,
                                     bias=0.0, scale=1.0)
                state[i] = {"qb": qb}

            def phase_scores(i):
                """QT transposes (PE), products (DVE), S matmuls (PE), exp (ACT),
                softmax (DVE)."""
                st = state[i]
                qb = st["qb"]
                qbr = qb[:]

                # QT psum tiles; QT1/QT3 copied to SBUF (DVE may read only
                # one PSUM operand per instruction).
                qt = {}
                tag_of = {1: "qtA", 0: "qtB", 2: "qtC", 3: "qtA"}

                def do_qt(xi):
                    qt[xi] = zqt.tile([P, 1024], BF16, tag=tag_of[xi], name=f"qt{xi}")
                    for half in range(2):
                        for tk in range(4):
                            nc.tensor.transpose(
                                qt[xi][:, half * 512 + tk * 128 : half * 512 + tk * 128 + 128],
                                _ap(qbr, xi * NTOK * C + tk * C + half * 128,
                                    [qbr.ap[0], [1, 128]]),
                                ident[:],
                            )

                S = zs.tile([P, 512], F32, tag="S")

                def do_prod(q_ref, k_ref):
                    for half in range(2):
                        pr = pprod.tile([P, 2048], BF16, tag="prod", name="prod")
                        nc.vector.tensor_tensor(
                            out=_ap(pr[:], 0, [pr[:].ap[0], [1, 2048]]),
                            in0=_ap(q_ref, half * 512,
                                    [q_ref.ap[0], [128, 4], [0, 4], [1, 128]]),
                            in1=_ap(k_ref, half * 512,
                                    [k_ref.ap[0], [0, 4], [128, 4], [1, 128]]),
                            op=ALU.mult,
                        )
                        xi = do_prod.xi
                        for tk in range(4):
                            for s in range(4):
                                nc.tensor.matmul(
                                    S[:, xi * 128 + tk * 32 + s * 8 + half * 4
                                      : xi * 128 + tk * 32 + s * 8 + half * 4 + 4],
                                    pr[:, (tk * 4 + s) * 128 : (tk * 4 + s) * 128 + 128],
                                    ones4[:],
                                    start=True, stop=True,
                                )

                do_qt(1)
                qs1 = pqts.tile([P, 1024], BF16, tag="qs1")
                nc.scalar.activation(out=qs1[:], in_=qt[1][:], func=ACT.Copy,
                                     bias=0.0, scale=1.0)
                do_qt(0)
                do_prod.xi = 0
                do_prod(qt[0][:], qs1[:])
                do_qt(3)
                qs3 = pqts.tile([P, 1024], BF16, tag="qs3")
                nc.scalar.activation(out=qs3[:], in_=qt[3][:], func=ACT.Copy,
                                     bias=0.0, scale=1.0)
                do_qt(2)
                do_prod.xi = 1
                do_prod(qs1[:], qt[2][:])
                do_prod.xi = 2
                do_prod(qt[2][:], qs3[:])
                do_prod.xi = 3
                do_prod(qs3[:], qs1[:])


                e = psmall.tile([P, 512], BF16, tag="e", bufs=3)
                nc.scalar.activation(out=e[:], in_=S[:], func=ACT.Exp,
                                     bias=0.0, scale=INV_SQRT_D)
                er = e[:]
                z1 = psmall.tile([P, 256], BF16, tag="z1", bufs=1)
                nc.vector.tensor_tensor(
                    out=z1[:],
                    in0=_ap(er, 0, [er.ap[0], [32, 16], [8, 2], [1, 8]]),
                    in1=_ap(er, 16, [er.ap[0], [32, 16], [8, 2], [1, 8]]),
                    op=ALU.add,
                )
                z1r = z1[:]
                z = psmall.tile([P, 128], F32, tag="z", bufs=1)
                nc.vector.tensor_tensor(
                    out=z[:],
                    in0=_ap(z1r, 0, [z1r.ap[0], [16, 16], [1, 8]]),
                    in1=_ap(z1r, 8, [z1r.ap[0], [16, 16], [1, 8]]),
                    op=ALU.add,
                )
                rz = psmall.tile([P, 128], F32, tag="rz", bufs=1)
                nc.vector.reciprocal(out=rz[:], in_=z[:])
                rzr = rz[:]
                pn = psmall.tile([P, 512], BF16, tag="pn", bufs=3)
                nc.vector.tensor_tensor(
                    out=pn[:],
                    in0=e[:],
                    in1=_ap(rzr, 0, [rzr.ap[0], [8, 16], [0, 4], [1, 8]]),
                    op=ALU.mult,
                )
                st["pn"] = pn
                st["e"] = e

            def phase_av(i):
                """pnT/kbT transposes (PE), kbT sbuf copy (Pool), pvT products
                (DVE), s-reduce + residual matmuls (PE), convert (ACT)."""
                st = state[i]
                qbr = st["qb"][:]
                pnr = st["pn"][:]

                xsq = pxsq.tile([P, 2 * FQ], BF16, tag="xsq")

                kbs = {}
                for xi in range(4):
                    # kbT transposes for this xi
                    kbt = zkbt.tile([P, 1024], BF16, tag="kbt", name=f"kbt{xi}")
                    for d8 in range(8):
                        nc.tensor.transpose(
                            kbt[:, d8 * 128 : d8 * 128 + 128],
                            _ap(qbr, xi * NTOK * C + d8,
                                [qbr.ap[0], [8, 128]]),
                            ident[:],
                        )
                    kb_sb = pkbs.tile([P, 1024], BF16, tag=f"kbs{xi}")
                    nc.scalar.activation(out=kb_sb[:], in_=kbt[:], func=ACT.Copy,
                                         bias=0.0, scale=1.0)
                    kbs[xi] = kb_sb

                # pn replicated over d4 (Pool): free order (xi,t,s,h,d4)
                pn_rep = psmall.tile([P, 2048], BF16, tag="pn_rep")
                for xi_ in range(4):
                    nc.scalar.activation(
                        out=pn_rep[:, xi_ * 512 : xi_ * 512 + 512],
                        in_=_ap(pnr, xi_ * 128,
                                [pnr.ap[0], [32, 4], [8, 4], [1, 8], [0, 4]]),
                        func=ACT.Copy, bias=0.0, scale=1.0,
                    )
                pn_rep_r = pn_rep[:]
                for xi in range(4):
                    pnt = zpnt.tile([P, 512], BF16, tag="pnt")
                    for tk in range(4):
                        nc.tensor.transpose(
                            pnt[:, tk * 128 : tk * 128 + 128],
                            _ap(pn_rep_r, (xi * 4 + tk) * 128,
                                [pn_rep_r.ap[0], [1, 128]]),
                            ident[:],
                        )
                    # AV products
                    pvt = ppvt.tile([P, FQ], BF16, tag="pvt", name=f"pvt{xi}")
                    pntr = pnt[:]
                    kbr = kbs[XK[xi]][:]
                    nc.vector.tensor_tensor(
                        out=_ap(pvt[:], 0, [pvt[:].ap[0], [1, FQ]]),
                        in0=_ap(pntr, 0, [pntr.ap[0], [128, 4], [0, 8], [1, 128]]),
                        in1=_ap(kbr, 0, [kbr.ap[0], [0, 4], [128, 8], [1, 128]]),
                        op=ALU.mult,
                    )
                    # s-reduce + residual matmuls into avout
                    av = zav.tile([P, 1024], F32, tag="av")
                    avr = av[:]
                    qb_sb = kbs[xi][:]
                    for tk in range(4):
                        for d8 in range(8):
                            o_ap = _ap(avr, tk * 256 + d8, [avr.ap[0], [8, 32]])
                            if AV_RESID:
                                nc.tensor.matmul(
                                    o_ap,
                                    _ap(qb_sb, d8 * 128, [qb_sb.ap[0], [1, 128]]),
                                    ident[:, 32 * tk : 32 * tk + 32],
                                    start=True, stop=not AV_ATT,
                                )
                            if AV_ATT:
                                nc.tensor.matmul(
                                    o_ap,
                                    pvt[:, (tk * 8 + d8) * 128 : (tk * 8 + d8) * 128 + 128],
                                    bd4[:],
                                    start=not AV_RESID, stop=True,
                                )
                    nc.scalar.activation(
                        out=xsq[:, xi * 1024 : xi * 1024 + 1024],
                        in_=av[:],
                        func=ACT.Copy, bias=0.0, scale=1.0,
                    )
                st["xsq"] = xsq

            def phase_ln(i):
                """sq (ACT), stats tree (DVE), LN scalars, normalize, store."""
                st = state[i]
                xsq = st["xsq"]
                xr = xsq[:]
                nc.scalar.activation(
                    out=_ap(xr, FQ, [xr.ap[0], [1, FQ]]),
                    in_=_ap(xr, 0, [xr.ap[0], [1, FQ]]),
                    func=ACT.Square, bias=0.0, scale=1.0,
                )
                xtr = pxtr.tile([P, FQ], BF16, tag="xtr")
                xtrr = xtr[:]
                # L1: 256->128 per group (32 groups)
                nc.vector.tensor_tensor(
                    out=_ap(xtrr, 0, [xtrr.ap[0], [128, 32], [1, 128]]),
                    in0=_ap(xr, 0, [xr.ap[0], [256, 32], [1, 128]]),
                    in1=_ap(xr, 128, [xr.ap[0], [256, 32], [1, 128]]),
                    op=ALU.add,
                )
                wdt = 64
                while wdt >= 2:
                    nc.vector.tensor_tensor(
                        out=_ap(xtrr, 0, [xtrr.ap[0], [128, 32], [1, wdt]]),
                        in0=_ap(xtrr, 0, [xtrr.ap[0], [128, 32], [1, wdt]]),
                        in1=_ap(xtrr, wdt, [xtrr.ap[0], [128, 32], [1, wdt]]),
                        op=ALU.add,
                    )
                    wdt //= 2
                stats = psmall.tile([P, 32], F32, tag="stats", bufs=1)
                nc.vector.tensor_tensor(
                    out=stats[:],
                    in0=_ap(xtrr, 0, [xtrr.ap[0], [128, 32], [1, 1]]),
                    in1=_ap(xtrr, 1, [xtrr.ap[0], [128, 32], [1, 1]]),
                    op=ALU.add,
                )
                ms = psmall.tile([P, 32], F32, tag="ms")
                nc.scalar.activation(out=ms[:], in_=stats[:], func=ACT.Copy,
                                     bias=0.0, scale=1.0 / C)
                musq = psmall.tile([P, 16], F32, tag="musq", bufs=1)
                nc.vector.tensor_tensor(
                    out=musq[:], in0=ms[:, :16], in1=ms[:, :16], op=ALU.mult
                )
                vpe = psmall.tile([P, 16], F32, tag="vpe", bufs=1)
                nc.vector.tensor_tensor(
                    out=vpe[:], in0=ms[:, 16:], in1=musq[:], op=ALU.subtract
                )
                sqv = psmall.tile([P, 16], F32, tag="sqv", bufs=1)
                nc.scalar.activation(out=sqv[:], in_=vpe[:], func=ACT.Sqrt,
                                     bias=epst[:], scale=1.0)
                rs = psmall.tile([P, 16], F32, tag="rs")
                nc.vector.reciprocal(out=rs[:], in_=sqv[:])

                y1 = py.tile([P, FQ], BF16, tag="y1")
                for k in range(16):
                    nc.vector.tensor_scalar(
                        out=y1[:, k * 256 : k * 256 + 256],
                        in0=xsq[:, k * 256 : k * 256 + 256],
                        scalar1=ms[:, k : k + 1],
                        scalar2=rs[:, k % 16 : k % 16 + 1],
                        op0=ALU.subtract,
                        op1=ALU.mult,
                    )
                wbr = wb[:]
                nc.vector.tensor_tensor(
                    out=y1[:],
                    in0=y1[:],
                    in1=_ap(wbr, 0, [wbr.ap[0], [C, 4], [0, 4], [1, C]]),
                    op=ALU.mult,
                )
                y2 = y1
                y = py.tile([P, FQ], F32, tag="y")
                if DEBUG_DUMP is not None:
                    nc.vector.memset(y[:], 0.0)
                    if DEBUG_DUMP == "sq":
                        nc.scalar.activation(out=y[:], in_=xsq[:, FQ:],
                                             func=ACT.Copy, bias=0.0, scale=1.0)
                    else:
                        dmp = {"e": st.get("e"), "pn": st.get("pn"),
                               "xres": xsq, "stats": stats, "y1": y1}[DEBUG_DUMP]
                        w_ = min(dmp.shape[1], FQ)
                        nc.scalar.activation(out=y[:, :w_], in_=dmp[:][:, :w_],
                                             func=ACT.Copy, bias=0.0, scale=1.0)
                yr = y[:]
                br = bcat[:]
                if DEBUG_DUMP is None:
                    nc.gpsimd.tensor_tensor(
                        out=_ap(yr, 0, [yr.ap[0], [C, 4], [NX * C, 4], [1, C]]),
                        in0=y2[:],
                        in1=_ap(br, 0, [br.ap[0], [C, 4], [0, 4], [1, C]]),
                        op=ALU.add,
                    )
                t = i % NTILES
                for qh in range(2):
                    dst = _ap(
                        out_ap,
                        (8 * t + qh) * W * NX * C,
                        [[2 * W * NX * C, 4], [2 * NX * C, NWI], [1, 2 * NX * C]],
                    )
                    src = _ap(yr, qh * 2 * NX * C, [yr.ap[0], [1, 2 * NX * C]])
                    nc.sync.dma_start(out=dst, in_=src)
                del state[i]

            # ---- software pipeline: stagger by one tile
            phase_load(0)
            for i in range(NT):
                if i + 1 < NT:
                    phase_load(i + 1)
                phase_scores(i)
                phase_av(i)
                if i >= 1:
                    phase_ln(i - 1)
            phase_ln(NT - 1)
    return nc


def _split_multi_waits(nc):
    wid = 0
    for fn in nc.m.functions:
        for blk in fn.blocks:
            new_list = []
            changed = False
            for inst in blk.instructions:
                si = inst.sync_info
                if si is not None:
                    waits = list(si.on_wait)
                    if len(waits) > 1:
                        for w in waits[:-1]:
                            ev = mybir.InstEventSemaphore(
                                name=f"WSPLIT-{wid}", ins=[], outs=[]
                            )
                            wid += 1
                            ev.engine = inst.engine
                            ev.sync_info = bass_rust.SyncInfo(on_wait=[w], on_update=[])
                            new_list.append(ev)
                        inst.sync_info = bass_rust.SyncInfo(
                            on_wait=[waits[-1]], on_update=list(si.on_update)
                        )
                        changed = True
                new_list.append(inst)
            if changed:
                blk.instructions = new_list


_NC_CACHE = None


def _get_nc():
    global _NC_CACHE
    if _NC_CACHE is None:
        nc = build_kernel()
        _split_multi_waits(nc)
        _NC_CACHE = nc
    return _NC_CACHE


def kernel(**inputs) -> np.ndarray:
    nc = _get_nc()
    param_names = [f"ln{a + 1}_{s}" for a in range(4) for s in ("w", "b")]
    in_maps = []
    for ci in range(B):
        m = {
            name: np.ascontiguousarray(np.asarray(inputs[name])[ci], dtype=np.float32)
            for name in ("r", "g", "b", "ir")
        }
        for pnm in param_names:
            m[pnm] = np.ascontiguousarray(np.asarray(inputs[pnm]), dtype=np.float32)
        in_maps.append(m)
    try:
        res = run_bass_kernel_spmd(nc, in_maps, list(range(B)))
    except ModuleNotFoundError:
        import os

        os.environ["BASS_NEVER_TRACE"] = "1"
        res = run_bass_kernel_spmd(nc, in_maps, list(range(B)))
    return np.stack([res.results[ci]["out"] for ci in range(B)], axis=0)


if __name__ == "__main__":
    from concourse.timeline_sim import TimelineSim

    nc = build_kernel()
    _split_multi_waits(nc)
    t = TimelineSim(nc).simulate()
    print(f"TimelineSim: {t:.0f} ns")


# revision 3
# speedup vs baseline: 1.0169x; 1.0169x over previous
"""Trainium2 Bass kernel for CAttentionBlock — v6: PE-offloaded reductions.

Layout per tile of 128 windows (all 4 attentions):
  qb   [128w, (xi,tok,c)=4096] bf16, natural channel order c=h*32+d
  QT   (PE transpose) [c128, (half,t,w)] bf16 psum, per xi
  prod (DVE) [c128, (t,s,w)=2048] bf16 sbuf, per (xi,half)
  S    (PE mm: lhsT=prod chunk, rhs=ones4 blockdiag) [128w, (xi,t,s,h8)=512] f32 psum
  e/pn (ACT exp + DVE softmax) [128w, 512] bf16
  pnT  (PE transpose w/ stride-0 d4 replication) [(s,h,d4)128, (t,w)] bf16 psum, per xi
  kbT  (PE transpose) [(tok,h,d4)128, (d8,w)] bf16 psum -> sbuf copy (Pool), per xi
  pvT  (DVE) [(s,h,d4)128, (t,d8,w)=4096] bf16 sbuf, per xi
  avout(PE mm pairs: s-reduce via bd4 + residual via identity slice)
       [128w, (t,c)=1024] f32 psum, per xi
  xres (ACT copy) -> xsq [128w, (xi,t,c)] bf16 sbuf; sq (ACT Square)
  stats (DVE halving tree) -> mu/rs (ACT+DVE tiny)
  norm: 16x tensor_scalar (x-mu)*rs (4x mode), *w (DVE TT), +b -> f32 (Pool)
"""

import sys

for _p in ("/opt/trn_rl_repo",):
    if _p not in sys.path:
        sys.path.insert(0, _p)

import numpy as np

import bass_rust
import concourse.bass as bass
import concourse.tile as tile
from concourse import mybir
from concourse.bass_utils import run_bass_kernel_spmd
from concourse.masks import make_identity

F32 = mybir.dt.float32
BF16 = mybir.dt.bfloat16
ALU = mybir.AluOpType
ACT = mybir.ActivationFunctionType
AX = mybir.AxisListType

B, H, W, C = 8, 64, 64, 256
WS = 2
NH = 8
D = C // NH            # 32
NTOK = WS * WS         # 4
NW = (H // WS) * (W // WS)
NWI = H // WS          # 32
P = 128
NTILES = NW // P       # 8
NX = 4
EPS = 1e-5
INV_SQRT_D = 1.0 / float(np.sqrt(D))
XK = [1, 2, 3, 1]      # K/V input index per attention

FQ = NX * NTOK * C         # 4096
DEBUG_DUMP = None  # "e" | "pn" | "xres" | "stats" | "y1"
AV_RESID = True
AV_ATT = True


def _ap(ref, offset_delta, dims):
    return bass_rust.AP(ref.tensor, ref.offset + offset_delta, [list(d) for d in dims])


def build_kernel(reps=1):
    nc = bass.Bass("TRN2", target_bir_lowering=False, debug=False)

    ins = {
        name: nc.dram_tensor(name, [H, W, C], F32, kind="ExternalInput")
        for name in ("r", "g", "b", "ir")
    }
    ln_params = []
    for a in range(4):
        wv = nc.dram_tensor(f"ln{a + 1}_w", [C], F32, kind="ExternalInput")
        bv = nc.dram_tensor(f"ln{a + 1}_b", [C], F32, kind="ExternalInput")
        ln_params.append((wv, bv))
    out = nc.dram_tensor("out", [H, W, 4 * C], F32, kind="ExternalOutput")

    in_aps = [ins[n].ap() for n in ("r", "g", "b", "ir")]
    out_ap = out.ap()
    NT = NTILES * reps

    with tile.TileContext(nc) as tc:
        with (
            tc.tile_pool(name="const", bufs=1) as pconst,
            tc.tile_pool(name="pin", bufs=2) as pin,
            tc.tile_pool(name="pqb", bufs=2) as pqb,
            tc.tile_pool(name="pprod", bufs=3) as pprod,
            tc.tile_pool(name="pqts", bufs=2) as pqts,
            tc.tile_pool(name="ppvt", bufs=2) as ppvt,
            tc.tile_pool(name="pkbs", bufs=1) as pkbs,
            tc.tile_pool(name="pxsq", bufs=2) as pxsq,
            tc.tile_pool(name="pxtr", bufs=1) as pxtr,
            tc.tile_pool(name="py", bufs=2) as py,
            tc.tile_pool(name="psmall", bufs=2) as psmall,
            tc.tile_pool(name="zqt", bufs=1, space="PSUM") as zqt,
            tc.tile_pool(name="zpnt", bufs=1, space="PSUM") as zpnt,
            tc.tile_pool(name="zkbt", bufs=1, space="PSUM") as zkbt,
            tc.tile_pool(name="zs", bufs=1, space="PSUM") as zs,
            tc.tile_pool(name="zav", bufs=1, space="PSUM") as zav,
        ):
            # ---------------- constants (DMAs deferred past tile-0 loads) ----
            wcat = pconst.tile([P, NX * C], F32, tag="wcat")
            bcat = pconst.tile([P, NX * C], F32, tag="bcat")
            epst = pconst.tile([P, 1], F32, tag="epst")
            nc.vector.memset(epst[:], EPS)
            wb = pconst.tile([P, NX * C], BF16, tag="wb")

            def load_consts():
                for a, (wv, bv) in enumerate(ln_params):
                    nc.sync.dma_start(
                        out=wcat[:, a * C : (a + 1) * C],
                        in_=_ap(wv.ap(), 0, [[0, P], [1, C]]),
                    )
                    nc.sync.dma_start(
                        out=bcat[:, a * C : (a + 1) * C],
                        in_=_ap(bv.ap(), 0, [[0, P], [1, C]]),
                    )
                nc.scalar.activation(out=wb[:], in_=wcat[:], func=ACT.Copy,
                                     bias=0.0, scale=1.0)

            ident = pconst.tile([P, P], BF16, tag="ident")
            make_identity(nc, ident[:])

            # ones4: [ (h4,d):128, 4 ]  1 at row p, col p//32
            ones4 = pconst.tile([P, 4], BF16, tag="ones4")
            nc.gpsimd.memset(ones4[:], 0.0)
            for h4 in range(4):
                nc.gpsimd.memset(ones4[32 * h4 : 32 * (h4 + 1), h4 : h4 + 1], 1.0)

            # bd4: [ (s,j):128, 32 ]  1 at col j = p mod 32 (identity32 per s block)
            bd4 = pconst.tile([P, 32], BF16, tag="bd4")
            nc.gpsimd.memset(bd4[:], 0.0)
            for s in range(4):
                make_identity(nc, bd4[32 * s : 32 * (s + 1), :], nomemset=True)

            # ---------------- per tile ----------------
            state = {}

            def phase_load(i):
                t = i % NTILES
                qcat = pin.tile([P, FQ], F32, tag="qcat")
                qr = qcat[:]
                for xi in range(4):
                    for qh in range(2):
                        src = _ap(
                            in_aps[xi],
                            (8 * t + qh) * W * C,
                            [[2 * W * C, 4], [2 * C, NWI], [1, 2 * C]],
                        )
                        dst = _ap(
                            qr, xi * NTOK * C + qh * 2 * C, [qr.ap[0], [1, 2 * C]]
                        )
                        nc.sync.dma_start(out=dst, in_=src)
                qb = pqb.tile([P, FQ], BF16, tag="qb")
                nc.scalar.activation(out=qb[:], in_=qcat[:], func=ACT.Copy,
                                     bias=0.0, scale=1.0)
                state[i] = {"qb": qb}

            def phase_scores(i):
                """QT transposes (PE), products (DVE), S matmuls (PE), exp (ACT),
                softmax (DVE)."""
                st = state[i]
                qb = st["qb"]
                qbr = qb[:]

                # QT psum tiles; QT1/QT3 copied to SBUF (DVE may read only
                # one PSUM operand per instruction).
                qt = {}
                tag_of = {1: "qtA", 0: "qtB", 2: "qtC", 3: "qtA"}

                def do_qt(xi):
                    qt[xi] = zqt.tile([P, 1024], BF16, tag=tag_of[xi], name=f"qt{xi}")
                    for half in range(2):
                        for tk in range(4):
                            nc.tensor.transpose(
                                qt[xi][:, half * 512 + tk * 128 : half * 512 + tk * 128 + 128],
                                _ap(qbr, xi * NTOK * C + tk * C + half * 128,
                                    [qbr.ap[0], [1, 128]]),
                                ident[:],
                            )

                S = zs.tile([P, 512], F32, tag="S")

                def do_prod(q_ref, k_ref):
                    for half in range(2):
                        pr = pprod.tile([P, 2048], BF16, tag="prod", name="prod")
                        nc.vector.tensor_tensor(
                            out=_ap(pr[:], 0, [pr[:].ap[0], [1, 2048]]),
                            in0=_ap(q_ref, half * 512,
                                    [q_ref.ap[0], [128, 4], [0, 4], [1, 128]]),
                            in1=_ap(k_ref, half * 512,
                                    [k_ref.ap[0], [0, 4], [128, 4], [1, 128]]),
                            op=ALU.mult,
                        )
                        xi = do_prod.xi
                        for tk in range(4):
                            for s in range(4):
                                nc.tensor.matmul(
                                    S[:, xi * 128 + tk * 32 + s * 8 + half * 4
                                      : xi * 128 + tk * 32 + s * 8 + half * 4 + 4],
                                    pr[:, (tk * 4 + s) * 128 : (tk * 4 + s) * 128 + 128],
                                    ones4[:],
                                    start=True, stop=True,
                                )

                do_qt(1)
                qs1 = pqts.tile([P, 1024], BF16, tag="qs1")
                nc.scalar.activation(out=qs1[:], in_=qt[1][:], func=ACT.Copy,
                                     bias=0.0, scale=1.0)
                do_qt(0)
                do_prod.xi = 0
                do_prod(qt[0][:], qs1[:])
                do_qt(3)
                qs3 = pqts.tile([P, 1024], BF16, tag="qs3")
                nc.scalar.activation(out=qs3[:], in_=qt[3][:], func=ACT.Copy,
                                     bias=0.0, scale=1.0)
                do_qt(2)
                do_prod.xi = 1
                do_prod(qs1[:], qt[2][:])
                do_prod.xi = 2
                do_prod(qt[2][:], qs3[:])
                do_prod.xi = 3
                do_prod(qs3[:], qs1[:])


                e = psmall.tile([P, 512], BF16, tag="e", bufs=3)
                nc.scalar.activation(out=e[:], in_=S[:], func=ACT.Exp,
                                     bias=0.0, scale=INV_SQRT_D)
                er = e[:]
                z1 = psmall.tile([P, 256], BF16, tag="z1", bufs=1)
                nc.vector.tensor_tensor(
                    out=z1[:],
                    in0=_ap(er, 0, [er.ap[0], [32, 16], [8, 2], [1, 8]]),
                    in1=_ap(er, 16, [er.ap[0], [32, 16], [8, 2], [1, 8]]),
                    op=ALU.add,
                )
                z1r = z1[:]
                z = psmall.tile([P, 128], F32, tag="z", bufs=1)
                nc.vector.tensor_tensor(
                    out=z[:],
                    in0=_ap(z1r, 0, [z1r.ap[0], [16, 16], [1, 8]]),
                    in1=_ap(z1r, 8, [z1r.ap[0], [16, 16], [1, 8]]),
                    op=ALU.add,
                )
                rz = psmall.tile([P, 128], F32, tag="rz", bufs=1)
                nc.vector.reciprocal(out=rz[:], in_=z[:])
                rzr = rz[:]
                pn = psmall.tile([P, 512], BF16, tag="pn", bufs=3)
                nc.vector.tensor_tensor(
                    out=pn[:],
                    in0=e[:],
                    in1=_ap(rzr, 0, [rzr.ap[0], [8, 16], [0, 4], [1, 8]]),
                    op=ALU.mult,
                )
                st["pn"] = pn
                st["e"] = e

                kbs = {}
                for xi in range(4):
                    kbt = zkbt.tile([P, 1024], BF16, tag="kbt", name=f"kbt{xi}")
                    for d8 in range(8):
                        nc.tensor.transpose(
                            kbt[:, d8 * 128 : d8 * 128 + 128],
                            _ap(qbr, xi * NTOK * C + d8,
                                [qbr.ap[0], [8, 128]]),
                            ident[:],
                        )
                    kb_sb = pkbs.tile([P, 1024], BF16, tag=f"kbs{xi}")
                    nc.scalar.activation(out=kb_sb[:], in_=kbt[:], func=ACT.Copy,
                                         bias=0.0, scale=1.0)
                    kbs[xi] = kb_sb
                st["kbs"] = kbs

            def phase_av(i):
                """pnT/kbT transposes (PE), kbT sbuf copy (Pool), pvT products
                (DVE), s-reduce + residual matmuls (PE), convert (ACT)."""
                st = state[i]
                qbr = st["qb"][:]
                pnr = st["pn"][:]

                xsq = pxsq.tile([P, 2 * FQ], BF16, tag="xsq")

                kbs = st["kbs"]

                # pn replicated over d4 (Pool): free order (xi,t,s,h,d4)
                pn_rep = psmall.tile([P, 2048], BF16, tag="pn_rep")
                for xi_ in range(4):
                    nc.scalar.activation(
                        out=pn_rep[:, xi_ * 512 : xi_ * 512 + 512],
                        in_=_ap(pnr, xi_ * 128,
                                [pnr.ap[0], [32, 4], [8, 4], [1, 8], [0, 4]]),
                        func=ACT.Copy, bias=0.0, scale=1.0,
                    )
                pn_rep_r = pn_rep[:]
                for xi in range(4):
                    pnt = zpnt.tile([P, 512], BF16, tag="pnt")
                    for tk in range(4):
                        nc.tensor.transpose(
                            pnt[:, tk * 128 : tk * 128 + 128],
                            _ap(pn_rep_r, (xi * 4 + tk) * 128,
                                [pn_rep_r.ap[0], [1, 128]]),
                            ident[:],
                        )
                    # AV products
                    pvt = ppvt.tile([P, FQ], BF16, tag="pvt", name=f"pvt{xi}")
                    pntr = pnt[:]
                    kbr = kbs[XK[xi]][:]
                    nc.vector.tensor_tensor(
                        out=_ap(pvt[:], 0, [pvt[:].ap[0], [1, FQ]]),
                        in0=_ap(pntr, 0, [pntr.ap[0], [128, 4], [0, 8], [1, 128]]),
                        in1=_ap(kbr, 0, [kbr.ap[0], [0, 4], [128, 8], [1, 128]]),
                        op=ALU.mult,
                    )
                    # s-reduce + residual matmuls into avout
                    av = zav.tile([P, 1024], F32, tag="av")
                    avr = av[:]
                    qb_sb = kbs[xi][:]
                    for tk in range(4):
                        for d8 in range(8):
                            o_ap = _ap(avr, tk * 256 + d8, [avr.ap[0], [8, 32]])
                            if AV_RESID:
                                nc.tensor.matmul(
                                    o_ap,
                                    _ap(qb_sb, d8 * 128, [qb_sb.ap[0], [1, 128]]),
                                    ident[:, 32 * tk : 32 * tk + 32],
                                    start=True, stop=not AV_ATT,
                                )
                            if AV_ATT:
                                nc.tensor.matmul(
                                    o_ap,
                                    pvt[:, (tk * 8 + d8) * 128 : (tk * 8 + d8) * 128 + 128],
                                    bd4[:],
                                    start=not AV_RESID, stop=True,
                                )
                    nc.scalar.activation(
                        out=xsq[:, xi * 1024 : xi * 1024 + 1024],
                        in_=av[:],
                        func=ACT.Copy, bias=0.0, scale=1.0,
                    )
                st["xsq"] = xsq

            def phase_ln(i):
                """sq (ACT), stats tree (DVE), LN scalars, normalize, store."""
                st = state[i]
                xsq = st["xsq"]
                xr = xsq[:]
                nc.scalar.activation(
                    out=_ap(xr, FQ, [xr.ap[0], [1, FQ]]),
                    in_=_ap(xr, 0, [xr.ap[0], [1, FQ]]),
                    func=ACT.Square, bias=0.0, scale=1.0,
                )
                xtr = pxtr.tile([P, FQ], BF16, tag="xtr")
                xtrr = xtr[:]
                # L1: 256->128 per group (32 groups)
                nc.vector.tensor_tensor(
                    out=_ap(xtrr, 0, [xtrr.ap[0], [128, 32], [1, 128]]),
                    in0=_ap(xr, 0, [xr.ap[0], [256, 32], [1, 128]]),
                    in1=_ap(xr, 128, [xr.ap[0], [256, 32], [1, 128]]),
                    op=ALU.add,
                )
                wdt = 64
                while wdt >= 2:
                    nc.vector.tensor_tensor(
                        out=_ap(xtrr, 0, [xtrr.ap[0], [128, 32], [1, wdt]]),
                        in0=_ap(xtrr, 0, [xtrr.ap[0], [128, 32], [1, wdt]]),
                        in1=_ap(xtrr, wdt, [xtrr.ap[0], [128, 32], [1, wdt]]),
                        op=ALU.add,
                    )
                    wdt //= 2
                stats = psmall.tile([P, 32], F32, tag="stats", bufs=1)
                nc.vector.tensor_tensor(
                    out=stats[:],
                    in0=_ap(xtrr, 0, [xtrr.ap[0], [128, 32], [1, 1]]),
                    in1=_ap(xtrr, 1, [xtrr.ap[0], [128, 32], [1, 1]]),
                    op=ALU.add,
                )
                ms = psmall.tile([P, 32], F32, tag="ms")
                nc.scalar.activation(out=ms[:], in_=stats[:], func=ACT.Copy,
                                     bias=0.0, scale=1.0 / C)
                musq = psmall.tile([P, 16], F32, tag="musq", bufs=1)
                nc.vector.tensor_tensor(
                    out=musq[:], in0=ms[:, :16], in1=ms[:, :16], op=ALU.mult
                )
                vpe = psmall.tile([P, 16], F32, tag="vpe", bufs=1)
                nc.vector.tensor_tensor(
                    out=vpe[:], in0=ms[:, 16:], in1=musq[:], op=ALU.subtract
                )
                sqv = psmall.tile([P, 16], F32, tag="sqv", bufs=1)
                nc.scalar.activation(out=sqv[:], in_=vpe[:], func=ACT.Sqrt,
                                     bias=epst[:], scale=1.0)
                rs = psmall.tile([P, 16], F32, tag="rs")
                nc.vector.reciprocal(out=rs[:], in_=sqv[:])

                y1 = py.tile([P, FQ], BF16, tag="y1")
                for k in range(16):
                    nc.vector.tensor_scalar(
                        out=y1[:, k * 256 : k * 256 + 256],
                        in0=xsq[:, k * 256 : k * 256 + 256],
                        scalar1=ms[:, k : k + 1],
                        scalar2=rs[:, k % 16 : k % 16 + 1],
                        op0=ALU.subtract,
                        op1=ALU.mult,
                    )
                wbr = wb[:]
                nc.vector.tensor_tensor(
                    out=y1[:],
                    in0=y1[:],
                    in1=_ap(wbr, 0, [wbr.ap[0], [C, 4], [0, 4], [1, C]]),
                    op=ALU.mult,
                )
                y2 = y1
                y = py.tile([P, FQ], F32, tag="y")
                if DEBUG_DUMP is not None:
                    nc.vector.memset(y[:], 0.0)
                    if DEBUG_DUMP == "sq":
                        nc.scalar.activation(out=y[:], in_=xsq[:, FQ:],
                                             func=ACT.Copy, bias=0.0, scale=1.0)
                    else:
                        dmp = {"e": st.get("e"), "pn": st.get("pn"),
                               "xres": xsq, "stats": stats, "y1": y1}[DEBUG_DUMP]
                        w_ = min(dmp.shape[1], FQ)
                        nc.scalar.activation(out=y[:, :w_], in_=dmp[:][:, :w_],
                                             func=ACT.Copy, bias=0.0, scale=1.0)
                yr = y[:]
                br = bcat[:]
                if DEBUG_DUMP is None:
                    nc.gpsimd.tensor_tensor(
                        out=_ap(yr, 0, [yr.ap[0], [C, 4], [NX * C, 4], [1, C]]),
                        in0=y2[:],
                        in1=_ap(br, 0, [br.ap[0], [C, 4], [0, 4], [1, C]]),
                        op=ALU.add,
                    )
                t = i % NTILES
                for qh in range(2):
                    dst = _ap(
                        out_ap,
                        (8 * t + qh) * W * NX * C,
                        [[2 * W * NX * C, 4], [2 * NX * C, NWI], [1, 2 * NX * C]],
                    )
                    src = _ap(yr, qh * 2 * NX * C, [yr.ap[0], [1, 2 * NX * C]])
                    nc.sync.dma_start(out=dst, in_=src)
                del state[i]

            # ---- software pipeline: stagger by one tile
            phase_load(0)
            load_consts()
            for i in range(NT):
                if i + 1 < NT:
                    phase_load(i + 1)
                phase_scores(i)
                phase_av(i)
                if i >= 1:
                    phase_ln(i - 1)
            phase_ln(NT - 1)
    return nc


def _split_multi_waits(nc):
    wid = 0
    for fn in nc.m.functions:
        for blk in fn.blocks:
            new_list = []
            changed = False
            for inst in blk.instructions:
                si = inst.sync_info
                if si is not None:
                    waits = list(si.on_wait)
                    if len(waits) > 1:
                        for w in waits[:-1]:
                            ev = mybir.InstEventSemaphore(
                                name=f"WSPLIT-{wid}", ins=[], outs=[]
                            )
                            wid += 1
                            ev.engine = inst.engine
                            ev.sync_info = bass_rust.SyncInfo(on_wait=[w], on_update=[])
                            new_list.append(ev)
                        inst.sync_info = bass_rust.SyncInfo(
                            on_wait=[waits[-1]], on_update=list(si.on_update)
                        )
                        changed = True
                new_list.append(inst)
            if changed:
                blk.instructions = new_list


_NC_CACHE = None


def _get_nc():
    global _NC_CACHE
    if _NC_CACHE is None:
        nc = build_kernel()
        _split_multi_waits(nc)
        _NC_CACHE = nc
    return _NC_CACHE


def kernel(**inputs) -> np.ndarray:
    nc = _get_nc()
    param_names = [f"ln{a + 1}_{s}" for a in range(4) for s in ("w", "b")]
    in_maps = []
    for ci in range(B):
        m = {
            name: np.ascontiguousarray(np.asarray(inputs[name])[ci], dtype=np.float32)
            for name in ("r", "g", "b", "ir")
        }
        for pnm in param_names:
            m[pnm] = np.ascontiguousarray(np.asarray(inputs[pnm]), dtype=np.float32)
        in_maps.append(m)
    try:
        res = run_bass_kernel_spmd(nc, in_maps, list(range(B)))
    except ModuleNotFoundError:
        import os

        os.environ["BASS_NEVER_TRACE"] = "1"
        res = run_bass_kernel_spmd(nc, in_maps, list(range(B)))
    return np.stack([res.results[ci]["out"] for ci in range(B)], axis=0)


if __name__ == "__main__":
    from concourse.timeline_sim import TimelineSim

    nc = build_kernel()
    _split_multi_waits(nc)
    t = TimelineSim(nc).simulate()
    print(f"TimelineSim: {t:.0f} ns")


# revision 6
# speedup vs baseline: 1.0906x; 1.0725x over previous
"""Trainium2 Bass kernel for CAttentionBlock — v6: PE-offloaded reductions.

Layout per tile of 128 windows (all 4 attentions):
  qb   [128w, (xi,tok,c)=4096] bf16, natural channel order c=h*32+d
  QT   (PE transpose) [c128, (half,t,w)] bf16 psum, per xi
  prod (DVE) [c128, (t,s,w)=2048] bf16 sbuf, per (xi,half)
  S    (PE mm: lhsT=prod chunk, rhs=ones4 blockdiag) [128w, (xi,t,s,h8)=512] f32 psum
  e/pn (ACT exp + DVE softmax) [128w, 512] bf16
  pnT  (PE transpose w/ stride-0 d4 replication) [(s,h,d4)128, (t,w)] bf16 psum, per xi
  kbT  (PE transpose) [(tok,h,d4)128, (d8,w)] bf16 psum -> sbuf copy (Pool), per xi
  pvT  (DVE) [(s,h,d4)128, (t,d8,w)=4096] bf16 sbuf, per xi
  avout(PE mm pairs: s-reduce via bd4 + residual via identity slice)
       [128w, (t,c)=1024] f32 psum, per xi
  xres (ACT copy) -> xsq [128w, (xi,t,c)] bf16 sbuf; sq (ACT Square)
  stats (DVE halving tree) -> mu/rs (ACT+DVE tiny)
  norm: 16x tensor_scalar (x-mu)*rs (4x mode), *w (DVE TT), +b -> f32 (Pool)
"""

import sys

for _p in ("/opt/trn_rl_repo",):
    if _p not in sys.path:
        sys.path.insert(0, _p)

import numpy as np

import bass_rust
import concourse.bass as bass
import concourse.tile as tile
from concourse import mybir
from concourse.bass_utils import run_bass_kernel_spmd
from concourse.masks import make_identity

F32 = mybir.dt.float32
BF16 = mybir.dt.bfloat16
ALU = mybir.AluOpType
ACT = mybir.ActivationFunctionType
AX = mybir.AxisListType

B, H, W, C = 8, 64, 64, 256
WS = 2
NH = 8
D = C // NH            # 32
NTOK = WS * WS         # 4
NW = (H // WS) * (W // WS)
NWI = H // WS          # 32
P = 128
NTILES = NW // P       # 8
NX = 4
EPS = 1e-5
INV_SQRT_D = 1.0 / float(np.sqrt(D))
XK = [1, 2, 3, 1]      # K/V input index per attention

FQ = NX * NTOK * C         # 4096
DEBUG_DUMP = None  # "e" | "pn" | "xres" | "stats" | "y1"
AV_RESID = True
AV_ATT = True


def _ap(ref, offset_delta, dims):
    return bass_rust.AP(ref.tensor, ref.offset + offset_delta, [list(d) for d in dims])


def build_kernel(reps=1):
    nc = bass.Bass("TRN2", target_bir_lowering=False, debug=False)

    ins = {
        name: nc.dram_tensor(name, [H, W, C], F32, kind="ExternalInput")
        for name in ("r", "g", "b", "ir")
    }
    ln_params = []
    for a in range(4):
        wv = nc.dram_tensor(f"ln{a + 1}_w", [C], F32, kind="ExternalInput")
        bv = nc.dram_tensor(f"ln{a + 1}_b", [C], F32, kind="ExternalInput")
        ln_params.append((wv, bv))
    out = nc.dram_tensor("out", [H, W, 4 * C], F32, kind="ExternalOutput")

    in_aps = [ins[n].ap() for n in ("r", "g", "b", "ir")]
    out_ap = out.ap()
    NT = NTILES * reps

    with tile.TileContext(nc) as tc:
        with (
            tc.tile_pool(name="const", bufs=1) as pconst,
            tc.tile_pool(name="pin", bufs=2) as pin,
            tc.tile_pool(name="pqb", bufs=2) as pqb,
            tc.tile_pool(name="pprod", bufs=2) as pprod,
            tc.tile_pool(name="pqts", bufs=2) as pqts,
            tc.tile_pool(name="ppvt", bufs=2) as ppvt,
            tc.tile_pool(name="pkbs", bufs=1) as pkbs,
            tc.tile_pool(name="pxsq", bufs=2) as pxsq,
            tc.tile_pool(name="pxtr", bufs=1) as pxtr,
            tc.tile_pool(name="py", bufs=2) as py,
            tc.tile_pool(name="psmall", bufs=2) as psmall,
            tc.tile_pool(name="zqt", bufs=1, space="PSUM") as zqt,
            tc.tile_pool(name="zpnt", bufs=1, space="PSUM") as zpnt,
            tc.tile_pool(name="zkbt", bufs=1, space="PSUM") as zkbt,
            tc.tile_pool(name="zs", bufs=1, space="PSUM") as zs,
            tc.tile_pool(name="zav", bufs=1, space="PSUM") as zav,
        ):
            # ---------------- constants (DMAs deferred past tile-0 loads) ----
            wcat = pconst.tile([P, NX * C], F32, tag="wcat")
            bcat = pconst.tile([P, NX * C], F32, tag="bcat")
            epst = pconst.tile([P, 1], F32, tag="epst")
            nc.vector.memset(epst[:], EPS)
            wb = pconst.tile([P, NX * C], BF16, tag="wb")

            def load_consts():
                for a, (wv, bv) in enumerate(ln_params):
                    nc.sync.dma_start(
                        out=wcat[:, a * C : (a + 1) * C],
                        in_=_ap(wv.ap(), 0, [[0, P], [1, C]]),
                    )
                    nc.sync.dma_start(
                        out=bcat[:, a * C : (a + 1) * C],
                        in_=_ap(bv.ap(), 0, [[0, P], [1, C]]),
                    )
                nc.scalar.activation(out=wb[:], in_=wcat[:], func=ACT.Copy,
                                     bias=0.0, scale=1.0)

            ident = pconst.tile([P, P], BF16, tag="ident")
            make_identity(nc, ident[:])

            # ones4: [ (h4,d):128, 4 ]  1 at row p, col p//32
            ones4 = pconst.tile([P, 4], BF16, tag="ones4")
            nc.gpsimd.memset(ones4[:], 0.0)
            for h4 in range(4):
                nc.gpsimd.memset(ones4[32 * h4 : 32 * (h4 + 1), h4 : h4 + 1], 1.0)

            # bd4: [ (s,j):128, 32 ]  1 at col j = p mod 32 (identity32 per s block)
            bd4 = pconst.tile([P, 32], BF16, tag="bd4")
            nc.gpsimd.memset(bd4[:], 0.0)
            for s in range(4):
                make_identity(nc, bd4[32 * s : 32 * (s + 1), :], nomemset=True)

            # ---------------- per tile ----------------
            state = {}

            def phase_load(i):
                t = i % NTILES
                qcat = pin.tile([P, FQ], F32, tag="qcat")
                qr = qcat[:]
                for xi in range(4):
                    for qh in range(2):
                        src = _ap(
                            in_aps[xi],
                            (8 * t + qh) * W * C,
                            [[2 * W * C, 4], [2 * C, NWI], [1, 2 * C]],
                        )
                        dst = _ap(
                            qr, xi * NTOK * C + qh * 2 * C, [qr.ap[0], [1, 2 * C]]
                        )
                        nc.sync.dma_start(out=dst, in_=src)
                qb = pqb.tile([P, FQ], BF16, tag="qb")
                nc.scalar.activation(out=qb[:], in_=qcat[:], func=ACT.Copy,
                                     bias=0.0, scale=1.0)
                state[i] = {"qb": qb}

            def phase_scores(i):
                """QT transposes (PE), products (DVE), S matmuls (PE), exp (ACT),
                softmax (DVE)."""
                st = state[i]
                qb = st["qb"]
                qbr = qb[:]

                # QT psum tiles; QT1/QT3 copied to SBUF (DVE may read only
                # one PSUM operand per instruction).
                qt = {}
                tag_of = {1: "qtA", 0: "qtB", 2: "qtC", 3: "qtA"}

                def do_qt(xi):
                    qt[xi] = zqt.tile([P, 1024], BF16, tag=tag_of[xi], name=f"qt{xi}")
                    for half in range(2):
                        for tk in range(4):
                            nc.tensor.transpose(
                                qt[xi][:, half * 512 + tk * 128 : half * 512 + tk * 128 + 128],
                                _ap(qbr, xi * NTOK * C + tk * C + half * 128,
                                    [qbr.ap[0], [1, 128]]),
                                ident[:],
                            )

                S = zs.tile([P, 512], F32, tag="S")

                def do_prod(q_ref, k_ref, eng=None):
                    pr = pprod.tile([P, 4096], BF16, tag="prod", name="prod")
                    (eng or nc.vector).tensor_tensor(
                        out=_ap(pr[:], 0, [pr[:].ap[0], [1, 4096]]),
                        in0=_ap(q_ref, 0,
                                [q_ref.ap[0], [512, 2], [128, 4], [0, 4], [1, 128]]),
                        in1=_ap(k_ref, 0,
                                [k_ref.ap[0], [512, 2], [0, 4], [128, 4], [1, 128]]),
                        op=ALU.mult,
                    )
                    xi = do_prod.xi
                    for half in range(2):
                        for tk in range(4):
                            for s in range(4):
                                nc.tensor.matmul(
                                    S[:, xi * 128 + tk * 32 + s * 8 + half * 4
                                      : xi * 128 + tk * 32 + s * 8 + half * 4 + 4],
                                    pr[:, half * 2048 + (tk * 4 + s) * 128
                                      : half * 2048 + (tk * 4 + s) * 128 + 128],
                                    ones4[:],
                                    start=True, stop=True,
                                )

                do_qt(1)
                qs1 = pqts.tile([P, 1024], BF16, tag="qs1")
                nc.scalar.activation(out=qs1[:], in_=qt[1][:], func=ACT.Copy,
                                     bias=0.0, scale=1.0)
                do_qt(0)
                do_prod.xi = 0
                do_prod(qt[0][:], qs1[:])
                do_qt(3)
                qs3 = pqts.tile([P, 1024], BF16, tag="qs3")
                nc.scalar.activation(out=qs3[:], in_=qt[3][:], func=ACT.Copy,
                                     bias=0.0, scale=1.0)
                do_qt(2)
                do_prod.xi = 1
                do_prod(qs1[:], qt[2][:])
                do_prod.xi = 2
                do_prod(qt[2][:], qs3[:])
                do_prod.xi = 3
                do_prod(qs3[:], qs1[:])


                e = psmall.tile([P, 512], BF16, tag="e", bufs=3)
                nc.scalar.activation(out=e[:], in_=S[:], func=ACT.Exp,
                                     bias=0.0, scale=INV_SQRT_D)
                er = e[:]
                z1 = psmall.tile([P, 256], BF16, tag="z1", bufs=1)
                nc.vector.tensor_tensor(
                    out=z1[:],
                    in0=_ap(er, 0, [er.ap[0], [32, 16], [8, 2], [1, 8]]),
                    in1=_ap(er, 16, [er.ap[0], [32, 16], [8, 2], [1, 8]]),
                    op=ALU.add,
                )
                z1r = z1[:]
                z = psmall.tile([P, 128], F32, tag="z", bufs=1)
                nc.vector.tensor_tensor(
                    out=z[:],
                    in0=_ap(z1r, 0, [z1r.ap[0], [16, 16], [1, 8]]),
                    in1=_ap(z1r, 8, [z1r.ap[0], [16, 16], [1, 8]]),
                    op=ALU.add,
                )
                rz = psmall.tile([P, 128], F32, tag="rz", bufs=1)
                nc.vector.reciprocal(out=rz[:], in_=z[:])
                rzr = rz[:]
                pn = psmall.tile([P, 512], BF16, tag="pn", bufs=3)
                nc.vector.tensor_tensor(
                    out=pn[:],
                    in0=e[:],
                    in1=_ap(rzr, 0, [rzr.ap[0], [8, 16], [0, 4], [1, 8]]),
                    op=ALU.mult,
                )
                st["pn"] = pn
                st["e"] = e

                kbs = {}
                for xi in range(4):
                    kbt = zkbt.tile([P, 1024], BF16, tag="kbt", name=f"kbt{xi}")
                    for d8 in range(8):
                        nc.tensor.transpose(
                            kbt[:, d8 * 128 : d8 * 128 + 128],
                            _ap(qbr, xi * NTOK * C + d8,
                                [qbr.ap[0], [8, 128]]),
                            ident[:],
                        )
                    kb_sb = pkbs.tile([P, 1024], BF16, tag=f"kbs{xi}")
                    nc.scalar.activation(out=kb_sb[:], in_=kbt[:], func=ACT.Copy,
                                         bias=0.0, scale=1.0)
                    kbs[xi] = kb_sb
                st["kbs"] = kbs

            def phase_av(i):
                """pnT/kbT transposes (PE), kbT sbuf copy (Pool), pvT products
                (DVE), s-reduce + residual matmuls (PE), convert (ACT)."""
                st = state[i]
                qbr = st["qb"][:]
                pnr = st["pn"][:]

                xsq = pxsq.tile([P, FQ], BF16, tag="xsq")

                kbs = st["kbs"]

                # pn replicated over d4 (Pool): free order (xi,t,s,h,d4)
                pn_rep = psmall.tile([P, 2048], BF16, tag="pn_rep", bufs=1)
                for xi_ in range(4):
                    nc.gpsimd.tensor_copy(
                        out=pn_rep[:, xi_ * 512 : xi_ * 512 + 512],
                        in_=_ap(pnr, xi_ * 128,
                                [pnr.ap[0], [32, 4], [8, 4], [1, 8], [0, 4]]),
                    )
                pn_rep_r = pn_rep[:]
                for xi in range(4):
                    pnt = zpnt.tile([P, 512], BF16, tag="pnt")
                    for tk in range(4):
                        nc.tensor.transpose(
                            pnt[:, tk * 128 : tk * 128 + 128],
                            _ap(pn_rep_r, (xi * 4 + tk) * 128,
                                [pn_rep_r.ap[0], [1, 128]]),
                            ident[:],
                        )
                    # AV products
                    pvt = ppvt.tile([P, FQ], BF16, tag="pvt", name=f"pvt{xi}")
                    pntr = pnt[:]
                    kbr = kbs[XK[xi]][:]
                    nc.vector.tensor_tensor(
                        out=_ap(pvt[:], 0, [pvt[:].ap[0], [1, FQ]]),
                        in0=_ap(pntr, 0, [pntr.ap[0], [128, 4], [0, 8], [1, 128]]),
                        in1=_ap(kbr, 0, [kbr.ap[0], [0, 4], [128, 8], [1, 128]]),
                        op=ALU.mult,
                    )
                    # s-reduce + residual matmuls into avout
                    av = zav.tile([P, 1024], F32, tag="av")
                    avr = av[:]
                    qb_sb = kbs[xi][:]
                    for tk in range(4):
                        for d8 in range(8):
                            o_ap = _ap(avr, tk * 256 + d8, [avr.ap[0], [8, 32]])
                            if AV_RESID:
                                nc.tensor.matmul(
                                    o_ap,
                                    _ap(qb_sb, d8 * 128, [qb_sb.ap[0], [1, 128]]),
                                    ident[:, 32 * tk : 32 * tk + 32],
                                    start=True, stop=not AV_ATT,
                                )
                            if AV_ATT:
                                nc.tensor.matmul(
                                    o_ap,
                                    pvt[:, (tk * 8 + d8) * 128 : (tk * 8 + d8) * 128 + 128],
                                    bd4[:],
                                    start=not AV_RESID, stop=True,
                                )
                    nc.scalar.activation(
                        out=xsq[:, xi * 1024 : xi * 1024 + 1024],
                        in_=av[:],
                        func=ACT.Copy, bias=0.0, scale=1.0,
                    )
                st["xsq"] = xsq

            def phase_ln(i):
                """sq-sums via ACT accumulator, x-tree (DVE), LN scalars,
                normalize, store."""
                st = state[i]
                xsq = st["xsq"]
                xr = xsq[:]
                stats = psmall.tile([P, 32], F32, tag="stats", bufs=1)
                sqscr = psmall.tile([P, 256], BF16, tag="sqscr", bufs=1)
                for k in range(16):
                    nc.scalar.activation(
                        out=sqscr[:],
                        in_=xsq[:, k * 256 : k * 256 + 256],
                        func=ACT.Square, bias=0.0, scale=1.0,
                        accum_out=stats[:, 16 + k : 17 + k],
                    )
                xtr = pxtr.tile([P, 2048], BF16, tag="xtr")
                xtrr = xtr[:]
                # L1: 256->128 per group (16 groups, x only)
                nc.vector.tensor_tensor(
                    out=_ap(xtrr, 0, [xtrr.ap[0], [128, 16], [1, 128]]),
                    in0=_ap(xr, 0, [xr.ap[0], [256, 16], [1, 128]]),
                    in1=_ap(xr, 128, [xr.ap[0], [256, 16], [1, 128]]),
                    op=ALU.add,
                )
                wdt = 64
                while wdt >= 2:
                    nc.vector.tensor_tensor(
                        out=_ap(xtrr, 0, [xtrr.ap[0], [128, 16], [1, wdt]]),
                        in0=_ap(xtrr, 0, [xtrr.ap[0], [128, 16], [1, wdt]]),
                        in1=_ap(xtrr, wdt, [xtrr.ap[0], [128, 16], [1, wdt]]),
                        op=ALU.add,
                    )
                    wdt //= 2
                nc.vector.tensor_tensor(
                    out=stats[:, :16],
                    in0=_ap(xtrr, 0, [xtrr.ap[0], [128, 16], [1, 1]]),
                    in1=_ap(xtrr, 1, [xtrr.ap[0], [128, 16], [1, 1]]),
                    op=ALU.add,
                )
                ms = psmall.tile([P, 32], F32, tag="ms")
                nc.scalar.activation(out=ms[:], in_=stats[:], func=ACT.Copy,
                                     bias=0.0, scale=1.0 / C)
                musq = psmall.tile([P, 16], F32, tag="musq", bufs=1)
                nc.vector.tensor_tensor(
                    out=musq[:], in0=ms[:, :16], in1=ms[:, :16], op=ALU.mult
                )
                vpe = psmall.tile([P, 16], F32, tag="vpe", bufs=1)
                nc.vector.tensor_tensor(
                    out=vpe[:], in0=ms[:, 16:], in1=musq[:], op=ALU.subtract
                )
                sqv = psmall.tile([P, 16], F32, tag="sqv", bufs=1)
                nc.scalar.activation(out=sqv[:], in_=vpe[:], func=ACT.Sqrt,
                                     bias=epst[:], scale=1.0)
                rs = psmall.tile([P, 16], F32, tag="rs")
                nc.vector.reciprocal(out=rs[:], in_=sqv[:])

                y1 = py.tile([P, FQ], BF16, tag="y1")
                for k in range(16):
                    nc.vector.tensor_scalar(
                        out=y1[:, k * 256 : k * 256 + 256],
                        in0=xsq[:, k * 256 : k * 256 + 256],
                        scalar1=ms[:, k : k + 1],
                        scalar2=rs[:, k % 16 : k % 16 + 1],
                        op0=ALU.subtract,
                        op1=ALU.mult,
                    )
                wbr = wb[:]
                nc.vector.tensor_tensor(
                    out=y1[:],
                    in0=y1[:],
                    in1=_ap(wbr, 0, [wbr.ap[0], [C, 4], [0, 4], [1, C]]),
                    op=ALU.mult,
                )
                y2 = y1
                y = py.tile([P, FQ], F32, tag="y")
                if DEBUG_DUMP is not None:
                    nc.vector.memset(y[:], 0.0)
                    if False:
                        pass
                    else:
                        dmp = {"e": st.get("e"), "pn": st.get("pn"),
                               "xres": xsq, "stats": stats, "y1": y1}[DEBUG_DUMP]
                        w_ = min(dmp.shape[1], FQ)
                        nc.scalar.activation(out=y[:, :w_], in_=dmp[:][:, :w_],
                                             func=ACT.Copy, bias=0.0, scale=1.0)
                yr = y[:]
                br = bcat[:]
                if DEBUG_DUMP is None:
                    eng_b = nc.vector if i == NT - 1 else nc.gpsimd
                    eng_b.tensor_tensor(
                        out=_ap(yr, 0, [yr.ap[0], [C, 4], [NX * C, 4], [1, C]]),
                        in0=y2[:],
                        in1=_ap(br, 0, [br.ap[0], [C, 4], [0, 4], [1, C]]),
                        op=ALU.add,
                    )
                t = i % NTILES
                for qh in range(2):
                    dst = _ap(
                        out_ap,
                        (8 * t + qh) * W * NX * C,
                        [[2 * W * NX * C, 4], [2 * NX * C, NWI], [1, 2 * NX * C]],
                    )
                    src = _ap(yr, qh * 2 * NX * C, [yr.ap[0], [1, 2 * NX * C]])
                    nc.sync.dma_start(out=dst, in_=src)
                del state[i]

            # ---- software pipeline: stagger by one tile
            phase_load(0)
            load_consts()
            for i in range(NT):
                if i + 1 < NT:
                    phase_load(i + 1)
                phase_scores(i)
                phase_av(i)
                if i >= 1:
                    phase_ln(i - 1)
            phase_ln(NT - 1)
    return nc


def _split_multi_waits(nc):
    wid = 0
    for fn in nc.m.functions:
        for blk in fn.blocks:
            new_list = []
            changed = False
            for inst in blk.instructions:
                si = inst.sync_info
                if si is not None:
                    waits = list(si.on_wait)
                    if len(waits) > 1:
                        for w in waits[:-1]:
                            ev = mybir.InstEventSemaphore(
                                name=f"WSPLIT-{wid}", ins=[], outs=[]
                            )
                            wid += 1
                            ev.engine = inst.engine
                            ev.sync_info = bass_rust.SyncInfo(on_wait=[w], on_update=[])
                            new_list.append(ev)
                        inst.sync_info = bass_rust.SyncInfo(
                            on_wait=[waits[-1]], on_update=list(si.on_update)
                        )
                        changed = True
                new_list.append(inst)
            if changed:
                blk.instructions = new_list


_NC_CACHE = None


def _get_nc():
    global _NC_CACHE
    if _NC_CACHE is None:
        nc = build_kernel()
        _split_multi_waits(nc)
        _NC_CACHE = nc
    return _NC_CACHE


def kernel(**inputs) -> np.ndarray:
    nc = _get_nc()
    param_names = [f"ln{a + 1}_{s}" for a in range(4) for s in ("w", "b")]
    in_maps = []
    for ci in range(B):
        m = {
            name: np.ascontiguousarray(np.asarray(inputs[name])[ci], dtype=np.float32)
            for name in ("r", "g", "b", "ir")
        }
        for pnm in param_names:
            m[pnm] = np.ascontiguousarray(np.asarray(inputs[pnm]), dtype=np.float32)
        in_maps.append(m)
    try:
        res = run_bass_kernel_spmd(nc, in_maps, list(range(B)))
    except ModuleNotFoundError:
        import os

        os.environ["BASS_NEVER_TRACE"] = "1"
        res = run_bass_kernel_spmd(nc, in_maps, list(range(B)))
    return np.stack([res.results[ci]["out"] for ci in range(B)], axis=0)


if __name__ == "__main__":
    from concourse.timeline_sim import TimelineSim

    nc = build_kernel()
    _split_multi_waits(nc)
    t = TimelineSim(nc).simulate()
    print(f"TimelineSim: {t:.0f} ns")


# revision 8
# speedup vs baseline: 1.1421x; 1.0472x over previous
"""Trainium2 Bass kernel for CAttentionBlock — v6: PE-offloaded reductions.

Layout per tile of 128 windows (all 4 attentions):
  qb   [128w, (xi,tok,c)=4096] bf16, natural channel order c=h*32+d
  QT   (PE transpose) [c128, (half,t,w)] bf16 psum, per xi
  prod (DVE) [c128, (t,s,w)=2048] bf16 sbuf, per (xi,half)
  S    (PE mm: lhsT=prod chunk, rhs=ones4 blockdiag) [128w, (xi,t,s,h8)=512] f32 psum
  e/pn (ACT exp + DVE softmax) [128w, 512] bf16
  pnT  (PE transpose w/ stride-0 d4 replication) [(s,h,d4)128, (t,w)] bf16 psum, per xi
  kbT  (PE transpose) [(tok,h,d4)128, (d8,w)] bf16 psum -> sbuf copy (Pool), per xi
  pvT  (DVE) [(s,h,d4)128, (t,d8,w)=4096] bf16 sbuf, per xi
  avout(PE mm pairs: s-reduce via bd4 + residual via identity slice)
       [128w, (t,c)=1024] f32 psum, per xi
  xres (ACT copy) -> xsq [128w, (xi,t,c)] bf16 sbuf; sq (ACT Square)
  stats (DVE halving tree) -> mu/rs (ACT+DVE tiny)
  norm: 16x tensor_scalar (x-mu)*rs (4x mode), *w (DVE TT), +b -> f32 (Pool)
"""

import sys

for _p in ("/opt/trn_rl_repo",):
    if _p not in sys.path:
        sys.path.insert(0, _p)

import numpy as np

import bass_rust
import concourse.bass as bass
import concourse.tile as tile
from concourse import mybir
from concourse.bass_utils import run_bass_kernel_spmd
from concourse.masks import make_identity

F32 = mybir.dt.float32
BF16 = mybir.dt.bfloat16
ALU = mybir.AluOpType
ACT = mybir.ActivationFunctionType
AX = mybir.AxisListType

B, H, W, C = 8, 64, 64, 256
WS = 2
NH = 8
D = C // NH            # 32
NTOK = WS * WS         # 4
NW = (H // WS) * (W // WS)
NWI = H // WS          # 32
P = 128
NTILES = NW // P       # 8
NX = 4
EPS = 1e-5
INV_SQRT_D = 1.0 / float(np.sqrt(D))
XK = [1, 2, 3, 1]      # K/V input index per attention

FQ = NX * NTOK * C         # 4096
DEBUG_DUMP = None  # "e" | "pn" | "xres" | "stats" | "y1"
AV_RESID = True
AV_ATT = True


def _ap(ref, offset_delta, dims):
    return bass_rust.AP(ref.tensor, ref.offset + offset_delta, [list(d) for d in dims])


def build_kernel(reps=1):
    nc = bass.Bass("TRN2", target_bir_lowering=False, debug=False)

    ins = {
        name: nc.dram_tensor(name, [H, W, C], F32, kind="ExternalInput")
        for name in ("r", "g", "b", "ir")
    }
    ln_params = []
    for a in range(4):
        wv = nc.dram_tensor(f"ln{a + 1}_w", [C], F32, kind="ExternalInput")
        bv = nc.dram_tensor(f"ln{a + 1}_b", [C], F32, kind="ExternalInput")
        ln_params.append((wv, bv))
    out = nc.dram_tensor("out", [H, W, 4 * C], F32, kind="ExternalOutput")

    in_aps = [ins[n].ap() for n in ("r", "g", "b", "ir")]
    out_ap = out.ap()
    NT = NTILES * reps

    with tile.TileContext(nc) as tc:
        with (
            tc.tile_pool(name="const", bufs=1) as pconst,
            tc.tile_pool(name="pin", bufs=2) as pin,
            tc.tile_pool(name="pqb", bufs=2) as pqb,
            tc.tile_pool(name="pprod", bufs=2) as pprod,
            tc.tile_pool(name="pqts", bufs=2) as pqts,
            tc.tile_pool(name="ppvt", bufs=2) as ppvt,
            tc.tile_pool(name="pkbs", bufs=1) as pkbs,
            tc.tile_pool(name="pxsq", bufs=2) as pxsq,
            tc.tile_pool(name="pxtr", bufs=1) as pxtr,
            tc.tile_pool(name="py", bufs=2) as py,
            tc.tile_pool(name="psmall", bufs=2) as psmall,
            tc.tile_pool(name="zqt", bufs=1, space="PSUM") as zqt,
            tc.tile_pool(name="zpnt", bufs=1, space="PSUM") as zpnt,
            tc.tile_pool(name="zkbt", bufs=1, space="PSUM") as zkbt,
            tc.tile_pool(name="zs", bufs=1, space="PSUM") as zs,
            tc.tile_pool(name="zav", bufs=1, space="PSUM") as zav,
        ):
            # ---------------- constants (DMAs deferred past tile-0 loads) ----
            wcat = pconst.tile([P, NX * C], F32, tag="wcat")
            bcat = pconst.tile([P, NX * C], F32, tag="bcat")
            epst = pconst.tile([P, 1], F32, tag="epst")
            nc.vector.memset(epst[:], EPS)
            wb = pconst.tile([P, NX * C], BF16, tag="wb")

            def load_consts():
                for a, (wv, bv) in enumerate(ln_params):
                    nc.sync.dma_start(
                        out=wcat[:, a * C : (a + 1) * C],
                        in_=_ap(wv.ap(), 0, [[0, P], [1, C]]),
                    )
                    nc.sync.dma_start(
                        out=bcat[:, a * C : (a + 1) * C],
                        in_=_ap(bv.ap(), 0, [[0, P], [1, C]]),
                    )
                nc.scalar.activation(out=wb[:], in_=wcat[:], func=ACT.Copy,
                                     bias=0.0, scale=1.0)

            ident = pconst.tile([P, P], BF16, tag="ident")
            make_identity(nc, ident[:])

            # ones4: [ (h4,d):128, 4 ]  1 at row p, col p//32
            ones4 = pconst.tile([P, 4], BF16, tag="ones4")
            nc.gpsimd.memset(ones4[:], 0.0)
            for h4 in range(4):
                nc.gpsimd.memset(ones4[32 * h4 : 32 * (h4 + 1), h4 : h4 + 1], 1.0)

            # bd4: [ (s,j):128, 32 ]  1 at col j = p mod 32 (identity32 per s block)
            bd4 = pconst.tile([P, 32], BF16, tag="bd4")
            nc.gpsimd.memset(bd4[:], 0.0)
            for s in range(4):
                make_identity(nc, bd4[32 * s : 32 * (s + 1), :], nomemset=True)

            # ---------------- per tile ----------------
            state = {}

            def phase_load(i):
                t = i % NTILES
                qcat = pin.tile([P, FQ], F32, tag="qcat")
                qr = qcat[:]
                for xi in range(4):
                    for qh in range(2):
                        src = _ap(
                            in_aps[xi],
                            (8 * t + qh) * W * C,
                            [[2 * W * C, 4], [2 * C, NWI], [1, 2 * C]],
                        )
                        dst = _ap(
                            qr, xi * NTOK * C + qh * 2 * C, [qr.ap[0], [1, 2 * C]]
                        )
                        nc.sync.dma_start(out=dst, in_=src)
                qb = pqb.tile([P, FQ], BF16, tag="qb")
                nc.scalar.activation(out=qb[:, :2048], in_=qcat[:, :2048],
                                     func=ACT.Copy, bias=0.0, scale=1.0)
                nc.scalar.activation(out=qb[:, 2048:], in_=qcat[:, 2048:],
                                     func=ACT.Copy, bias=0.0, scale=1.0)
                state[i] = {"qb": qb}

            def phase_scores(i):
                """QT transposes (PE), products (DVE), S matmuls (PE), exp (ACT),
                softmax (DVE)."""
                st = state[i]
                qb = st["qb"]
                qbr = qb[:]

                # QT psum tiles; QT1/QT3 copied to SBUF (DVE may read only
                # one PSUM operand per instruction).
                qt = {}
                tag_of = {1: "qtA", 0: "qtB", 2: "qtC", 3: "qtA"}

                def do_qt(xi):
                    qt[xi] = zqt.tile([P, 1024], BF16, tag=tag_of[xi], name=f"qt{xi}")
                    for half in range(2):
                        for tk in range(4):
                            nc.tensor.transpose(
                                qt[xi][:, half * 512 + tk * 128 : half * 512 + tk * 128 + 128],
                                _ap(qbr, xi * NTOK * C + tk * C + half * 128,
                                    [qbr.ap[0], [1, 128]]),
                                ident[:],
                            )

                S = zs.tile([P, 512], F32, tag="S")

                def do_prod(q_ref, k_ref, eng=None):
                    pr = pprod.tile([P, 4096], BF16, tag="prod", name="prod")
                    (eng or nc.vector).tensor_tensor(
                        out=_ap(pr[:], 0, [pr[:].ap[0], [1, 4096]]),
                        in0=_ap(q_ref, 0,
                                [q_ref.ap[0], [512, 2], [128, 4], [0, 4], [1, 128]]),
                        in1=_ap(k_ref, 0,
                                [k_ref.ap[0], [512, 2], [0, 4], [128, 4], [1, 128]]),
                        op=ALU.mult,
                    )
                    xi = do_prod.xi
                    for half in range(2):
                        for tk in range(4):
                            for s in range(4):
                                nc.tensor.matmul(
                                    S[:, xi * 128 + tk * 32 + s * 8 + half * 4
                                      : xi * 128 + tk * 32 + s * 8 + half * 4 + 4],
                                    pr[:, half * 2048 + (tk * 4 + s) * 128
                                      : half * 2048 + (tk * 4 + s) * 128 + 128],
                                    ones4[:],
                                    start=True, stop=True,
                                )

                do_qt(1)
                qs1 = pqts.tile([P, 1024], BF16, tag="qs1")
                nc.scalar.activation(out=qs1[:], in_=qt[1][:], func=ACT.Copy,
                                     bias=0.0, scale=1.0)
                do_qt(0)
                do_prod.xi = 0
                do_prod(qt[0][:], qs1[:])
                do_qt(3)
                qs3 = pqts.tile([P, 1024], BF16, tag="qs3")
                nc.scalar.activation(out=qs3[:], in_=qt[3][:], func=ACT.Copy,
                                     bias=0.0, scale=1.0)
                do_qt(2)
                do_prod.xi = 1
                do_prod(qs1[:], qt[2][:])
                do_prod.xi = 2
                do_prod(qt[2][:], qs3[:])
                do_prod.xi = 3
                do_prod(qs3[:], qs1[:])


                e = psmall.tile([P, 512], BF16, tag="e", bufs=3)
                nc.scalar.activation(out=e[:], in_=S[:], func=ACT.Exp,
                                     bias=0.0, scale=INV_SQRT_D)
                er = e[:]
                z1 = psmall.tile([P, 256], BF16, tag="z1", bufs=1)
                nc.vector.tensor_tensor(
                    out=z1[:],
                    in0=_ap(er, 0, [er.ap[0], [32, 16], [8, 2], [1, 8]]),
                    in1=_ap(er, 16, [er.ap[0], [32, 16], [8, 2], [1, 8]]),
                    op=ALU.add,
                )
                z1r = z1[:]
                z = psmall.tile([P, 128], F32, tag="z", bufs=1)
                nc.vector.tensor_tensor(
                    out=z[:],
                    in0=_ap(z1r, 0, [z1r.ap[0], [16, 16], [1, 8]]),
                    in1=_ap(z1r, 8, [z1r.ap[0], [16, 16], [1, 8]]),
                    op=ALU.add,
                )
                rz = psmall.tile([P, 128], F32, tag="rz", bufs=1)
                nc.vector.reciprocal(out=rz[:], in_=z[:])
                rzr = rz[:]
                pn = psmall.tile([P, 512], BF16, tag="pn", bufs=3)
                nc.vector.tensor_tensor(
                    out=pn[:],
                    in0=e[:],
                    in1=_ap(rzr, 0, [rzr.ap[0], [8, 16], [0, 4], [1, 8]]),
                    op=ALU.mult,
                )
                st["pn"] = pn
                st["e"] = e

                kbs = {}
                for xi in range(4):
                    kbt = zkbt.tile([P, 1024], BF16, tag="kbt", name=f"kbt{xi}")
                    for d8 in range(8):
                        nc.tensor.transpose(
                            kbt[:, d8 * 128 : d8 * 128 + 128],
                            _ap(qbr, xi * NTOK * C + d8,
                                [qbr.ap[0], [8, 128]]),
                            ident[:],
                        )
                    kb_sb = pkbs.tile([P, 1024], BF16, tag=f"kbs{xi}")
                    nc.scalar.activation(out=kb_sb[:], in_=kbt[:], func=ACT.Copy,
                                         bias=0.0, scale=1.0)
                    kbs[xi] = kb_sb
                st["kbs"] = kbs

            def phase_av(i):
                """pnT/kbT transposes (PE), kbT sbuf copy (Pool), pvT products
                (DVE), s-reduce + residual matmuls (PE), convert (ACT)."""
                st = state[i]
                qbr = st["qb"][:]
                pnr = st["pn"][:]

                xsq = pxsq.tile([P, FQ], BF16, tag="xsq")

                kbs = st["kbs"]

                # pn replicated over d4 (Pool): free order (xi,t,s,h,d4)
                pn_rep = psmall.tile([P, 2048], BF16, tag="pn_rep", bufs=1)
                for xi_ in range(4):
                    nc.gpsimd.tensor_copy(
                        out=pn_rep[:, xi_ * 512 : xi_ * 512 + 512],
                        in_=_ap(pnr, xi_ * 128,
                                [pnr.ap[0], [32, 4], [8, 4], [1, 8], [0, 4]]),
                    )
                pn_rep_r = pn_rep[:]
                pnt_pair = {}
                for xi in range(4):
                    if xi % 2 == 0:
                        pnt_pair[xi // 2] = zpnt.tile(
                            [P, 1024], BF16, tag="pnt", name=f"pnt{xi // 2}")
                        for xj in (xi, xi + 1):
                            for tk in range(4):
                                nc.tensor.transpose(
                                    pnt_pair[xi // 2][:, (xj % 2) * 512 + tk * 128
                                                      : (xj % 2) * 512 + tk * 128 + 128],
                                    _ap(pn_rep_r, (xj * 4 + tk) * 128,
                                        [pn_rep_r.ap[0], [1, 128]]),
                                    ident[:],
                                )
                    pnt = pnt_pair[xi // 2][:, (xi % 2) * 512 : (xi % 2) * 512 + 512]
                    # AV products
                    pvt = ppvt.tile([P, FQ], BF16, tag="pvt", name=f"pvt{xi}")
                    pntr = pnt[:]
                    kbr = kbs[XK[xi]][:]
                    nc.vector.tensor_tensor(
                        out=_ap(pvt[:], 0, [pvt[:].ap[0], [1, FQ]]),
                        in0=_ap(pntr, 0, [pntr.ap[0], [128, 4], [0, 8], [1, 128]]),
                        in1=_ap(kbr, 0, [kbr.ap[0], [0, 4], [128, 8], [1, 128]]),
                        op=ALU.mult,
                    )
                    # s-reduce + residual matmuls into avout
                    av = zav.tile([P, 1024], F32, tag="av")
                    avr = av[:]
                    qb_sb = kbs[xi][:]
                    for tk in range(4):
                        for d8 in range(8):
                            o_ap = _ap(avr, tk * 256 + d8, [avr.ap[0], [8, 32]])
                            if AV_RESID:
                                nc.tensor.matmul(
                                    o_ap,
                                    _ap(qb_sb, d8 * 128, [qb_sb.ap[0], [1, 128]]),
                                    ident[:, 32 * tk : 32 * tk + 32],
                                    start=True, stop=not AV_ATT,
                                )
                            if AV_ATT:
                                nc.tensor.matmul(
                                    o_ap,
                                    pvt[:, (tk * 8 + d8) * 128 : (tk * 8 + d8) * 128 + 128],
                                    bd4[:],
                                    start=not AV_RESID, stop=True,
                                )
                    nc.scalar.activation(
                        out=xsq[:, xi * 1024 : xi * 1024 + 1024],
                        in_=av[:],
                        func=ACT.Copy, bias=0.0, scale=1.0,
                    )
                st["xsq"] = xsq

            def phase_ln(i):
                """sq-sums via ACT accumulator, x-tree (DVE), LN scalars,
                normalize, store."""
                st = state[i]
                xsq = st["xsq"]
                xr = xsq[:]
                stats = psmall.tile([P, 32], F32, tag="stats", bufs=1)
                sqscr = psmall.tile([P, 256], BF16, tag="sqscr", bufs=1)
                for k in range(16):
                    nc.scalar.activation(
                        out=sqscr[:],
                        in_=xsq[:, k * 256 : k * 256 + 256],
                        func=ACT.Square, bias=0.0, scale=1.0,
                        accum_out=stats[:, 16 + k : 17 + k],
                    )
                xtr = pxtr.tile([P, 2048], BF16, tag="xtr")
                xtrr = xtr[:]
                # L1: 256->128 per group (16 groups, x only)
                nc.vector.tensor_tensor(
                    out=_ap(xtrr, 0, [xtrr.ap[0], [128, 16], [1, 128]]),
                    in0=_ap(xr, 0, [xr.ap[0], [256, 16], [1, 128]]),
                    in1=_ap(xr, 128, [xr.ap[0], [256, 16], [1, 128]]),
                    op=ALU.add,
                )
                wdt = 64
                while wdt >= 2:
                    nc.vector.tensor_tensor(
                        out=_ap(xtrr, 0, [xtrr.ap[0], [128, 16], [1, wdt]]),
                        in0=_ap(xtrr, 0, [xtrr.ap[0], [128, 16], [1, wdt]]),
                        in1=_ap(xtrr, wdt, [xtrr.ap[0], [128, 16], [1, wdt]]),
                        op=ALU.add,
                    )
                    wdt //= 2
                nc.vector.tensor_tensor(
                    out=stats[:, :16],
                    in0=_ap(xtrr, 0, [xtrr.ap[0], [128, 16], [1, 1]]),
                    in1=_ap(xtrr, 1, [xtrr.ap[0], [128, 16], [1, 1]]),
                    op=ALU.add,
                )
                ms = psmall.tile([P, 32], F32, tag="ms")
                nc.scalar.activation(out=ms[:], in_=stats[:], func=ACT.Copy,
                                     bias=0.0, scale=1.0 / C)
                musq = psmall.tile([P, 16], F32, tag="musq", bufs=1)
                nc.vector.tensor_tensor(
                    out=musq[:], in0=ms[:, :16], in1=ms[:, :16], op=ALU.mult
                )
                vpe = psmall.tile([P, 16], F32, tag="vpe", bufs=1)
                nc.vector.tensor_tensor(
                    out=vpe[:], in0=ms[:, 16:], in1=musq[:], op=ALU.subtract
                )
                sqv = psmall.tile([P, 16], F32, tag="sqv", bufs=1)
                nc.scalar.activation(out=sqv[:], in_=vpe[:], func=ACT.Sqrt,
                                     bias=epst[:], scale=1.0)
                rs = psmall.tile([P, 16], F32, tag="rs")
                nc.vector.reciprocal(out=rs[:], in_=sqv[:])

                y1 = py.tile([P, FQ], BF16, tag="y1")
                for k in range(16):
                    nc.vector.tensor_scalar(
                        out=y1[:, k * 256 : k * 256 + 256],
                        in0=xsq[:, k * 256 : k * 256 + 256],
                        scalar1=ms[:, k : k + 1],
                        scalar2=rs[:, k % 16 : k % 16 + 1],
                        op0=ALU.subtract,
                        op1=ALU.mult,
                    )
                wbr = wb[:]
                nc.vector.tensor_tensor(
                    out=y1[:],
                    in0=y1[:],
                    in1=_ap(wbr, 0, [wbr.ap[0], [C, 4], [0, 4], [1, C]]),
                    op=ALU.mult,
                )
                y2 = y1
                st["y2"] = y2
                st["i"] = i

            def phase_ln_tail(i):
                st = state[i]
                y2 = st["y2"]
                xsq = st["xsq"]
                y1 = y2
                stats = None
                y = py.tile([P, FQ], F32, tag="y")
                if DEBUG_DUMP is not None:
                    nc.vector.memset(y[:], 0.0)
                    dmp = {"e": st.get("e"), "pn": st.get("pn"),
                           "xres": xsq}[DEBUG_DUMP]
                    w_ = min(dmp.shape[1], FQ)
                    nc.scalar.activation(out=y[:, :w_], in_=dmp[:][:, :w_],
                                         func=ACT.Copy, bias=0.0, scale=1.0)
                yr = y[:]
                br = bcat[:]
                if DEBUG_DUMP is None:
                    eng_b = nc.vector if st["i"] == NT - 1 else nc.gpsimd
                    eng_b.tensor_tensor(
                        out=_ap(yr, 0, [yr.ap[0], [C, 4], [NX * C, 4], [1, C]]),
                        in0=y2[:],
                        in1=_ap(br, 0, [br.ap[0], [C, 4], [0, 4], [1, C]]),
                        op=ALU.add,
                    )
                t = i % NTILES
                for qh in range(2):
                    dst = _ap(
                        out_ap,
                        (8 * t + qh) * W * NX * C,
                        [[2 * W * NX * C, 4], [2 * NX * C, NWI], [1, 2 * NX * C]],
                    )
                    src = _ap(yr, qh * 2 * NX * C, [yr.ap[0], [1, 2 * NX * C]])
                    nc.sync.dma_start(out=dst, in_=src)
                del state[i]

            # ---- software pipeline: stagger by one tile
            phase_load(0)
            load_consts()
            for i in range(NT):
                if i + 1 < NT:
                    phase_load(i + 1)
                phase_scores(i)
                phase_av(i)
                if i >= 1:
                    phase_ln(i - 1)
                if i >= 2:
                    phase_ln_tail(i - 2)
            phase_ln(NT - 1)
            phase_ln_tail(NT - 2)
            phase_ln_tail(NT - 1)
    return nc


def _split_multi_waits(nc):
    wid = 0
    for fn in nc.m.functions:
        for blk in fn.blocks:
            new_list = []
            changed = False
            for inst in blk.instructions:
                si = inst.sync_info
                if si is not None:
                    waits = list(si.on_wait)
                    if len(waits) > 1:
                        for w in waits[:-1]:
                            ev = mybir.InstEventSemaphore(
                                name=f"WSPLIT-{wid}", ins=[], outs=[]
                            )
                            wid += 1
                            ev.engine = inst.engine
                            ev.sync_info = bass_rust.SyncInfo(on_wait=[w], on_update=[])
                            new_list.append(ev)
                        inst.sync_info = bass_rust.SyncInfo(
                            on_wait=[waits[-1]], on_update=list(si.on_update)
                        )
                        changed = True
                new_list.append(inst)
            if changed:
                blk.instructions = new_list


_NC_CACHE = None


def _get_nc():
    global _NC_CACHE
    if _NC_CACHE is None:
        nc = build_kernel()
        _split_multi_waits(nc)
        _NC_CACHE = nc
    return _NC_CACHE


def kernel(**inputs) -> np.ndarray:
    nc = _get_nc()
    param_names = [f"ln{a + 1}_{s}" for a in range(4) for s in ("w", "b")]
    in_maps = []
    for ci in range(B):
        m = {
            name: np.ascontiguousarray(np.asarray(inputs[name])[ci], dtype=np.float32)
            for name in ("r", "g", "b", "ir")
        }
        for pnm in param_names:
            m[pnm] = np.ascontiguousarray(np.asarray(inputs[pnm]), dtype=np.float32)
        in_maps.append(m)
    try:
        res = run_bass_kernel_spmd(nc, in_maps, list(range(B)))
    except ModuleNotFoundError:
        import os

        os.environ["BASS_NEVER_TRACE"] = "1"
        res = run_bass_kernel_spmd(nc, in_maps, list(range(B)))
    return np.stack([res.results[ci]["out"] for ci in range(B)], axis=0)


if __name__ == "__main__":
    from concourse.timeline_sim import TimelineSim

    nc = build_kernel()
    _split_multi_waits(nc)
    t = TimelineSim(nc).simulate()
    print(f"TimelineSim: {t:.0f} ns")


# revision 9
# speedup vs baseline: 1.1547x; 1.0111x over previous
"""Trainium2 Bass kernel for CAttentionBlock — v6: PE-offloaded reductions.

Layout per tile of 128 windows (all 4 attentions):
  qb   [128w, (xi,tok,c)=4096] bf16, natural channel order c=h*32+d
  QT   (PE transpose) [c128, (half,t,w)] bf16 psum, per xi
  prod (DVE) [c128, (t,s,w)=2048] bf16 sbuf, per (xi,half)
  S    (PE mm: lhsT=prod chunk, rhs=ones4 blockdiag) [128w, (xi,t,s,h8)=512] f32 psum
  e/pn (ACT exp + DVE softmax) [128w, 512] bf16
  pnT  (PE transpose w/ stride-0 d4 replication) [(s,h,d4)128, (t,w)] bf16 psum, per xi
  kbT  (PE transpose) [(tok,h,d4)128, (d8,w)] bf16 psum -> sbuf copy (Pool), per xi
  pvT  (DVE) [(s,h,d4)128, (t,d8,w)=4096] bf16 sbuf, per xi
  avout(PE mm pairs: s-reduce via bd4 + residual via identity slice)
       [128w, (t,c)=1024] f32 psum, per xi
  xres (ACT copy) -> xsq [128w, (xi,t,c)] bf16 sbuf; sq (ACT Square)
  stats (DVE halving tree) -> mu/rs (ACT+DVE tiny)
  norm: 16x tensor_scalar (x-mu)*rs (4x mode), *w (DVE TT), +b -> f32 (Pool)
"""

import sys

for _p in ("/opt/trn_rl_repo",):
    if _p not in sys.path:
        sys.path.insert(0, _p)

import numpy as np

import bass_rust
import concourse.bass as bass
import concourse.tile as tile
from concourse import mybir
from concourse.bass_utils import run_bass_kernel_spmd
from concourse.masks import make_identity

F32 = mybir.dt.float32
BF16 = mybir.dt.bfloat16
ALU = mybir.AluOpType
ACT = mybir.ActivationFunctionType
AX = mybir.AxisListType

B, H, W, C = 8, 64, 64, 256
WS = 2
NH = 8
D = C // NH            # 32
NTOK = WS * WS         # 4
NW = (H // WS) * (W // WS)
NWI = H // WS          # 32
P = 128
NTILES = NW // P       # 8
NX = 4
EPS = 1e-5
INV_SQRT_D = 1.0 / float(np.sqrt(D))
XK = [1, 2, 3, 1]      # K/V input index per attention

FQ = NX * NTOK * C         # 4096
DEBUG_DUMP = None  # "e" | "pn" | "xres" | "stats" | "y1"
AV_RESID = True
AV_ATT = True


def _ap(ref, offset_delta, dims):
    return bass_rust.AP(ref.tensor, ref.offset + offset_delta, [list(d) for d in dims])


def build_kernel(reps=1):
    nc = bass.Bass("TRN2", target_bir_lowering=False, debug=False)

    ins = {
        name: nc.dram_tensor(name, [H, W, C], F32, kind="ExternalInput")
        for name in ("r", "g", "b", "ir")
    }
    ln_params = []
    for a in range(4):
        wv = nc.dram_tensor(f"ln{a + 1}_w", [C], F32, kind="ExternalInput")
        bv = nc.dram_tensor(f"ln{a + 1}_b", [C], F32, kind="ExternalInput")
        ln_params.append((wv, bv))
    out = nc.dram_tensor("out", [H, W, 4 * C], F32, kind="ExternalOutput")

    in_aps = [ins[n].ap() for n in ("r", "g", "b", "ir")]
    out_ap = out.ap()
    NT = NTILES * reps

    with tile.TileContext(nc) as tc:
        with (
            tc.tile_pool(name="const", bufs=1) as pconst,
            tc.tile_pool(name="pin", bufs=2) as pin,
            tc.tile_pool(name="pqb", bufs=2) as pqb,
            tc.tile_pool(name="pprod", bufs=2) as pprod,
            tc.tile_pool(name="pqts", bufs=2) as pqts,
            tc.tile_pool(name="ppvt", bufs=2) as ppvt,
            tc.tile_pool(name="pkbs", bufs=1) as pkbs,
            tc.tile_pool(name="pxsq", bufs=2) as pxsq,
            tc.tile_pool(name="pxtr", bufs=1) as pxtr,
            tc.tile_pool(name="py", bufs=2) as py,
            tc.tile_pool(name="psmall", bufs=2) as psmall,
            tc.tile_pool(name="zqt", bufs=1, space="PSUM") as zqt,
            tc.tile_pool(name="zpnt", bufs=1, space="PSUM") as zpnt,
            tc.tile_pool(name="zkbt", bufs=1, space="PSUM") as zkbt,
            tc.tile_pool(name="zs", bufs=1, space="PSUM") as zs,
            tc.tile_pool(name="zav", bufs=1, space="PSUM") as zav,
        ):
            # ---------------- constants (DMAs deferred past tile-0 loads) ----
            wcat = pconst.tile([P, NX * C], F32, tag="wcat")
            bcat = pconst.tile([P, NX * C], F32, tag="bcat")
            epst = pconst.tile([P, 1], F32, tag="epst")
            nc.vector.memset(epst[:], EPS)
            wb = pconst.tile([P, NX * C], BF16, tag="wb")

            def load_consts():
                for a, (wv, bv) in enumerate(ln_params):
                    nc.sync.dma_start(
                        out=wcat[:, a * C : (a + 1) * C],
                        in_=_ap(wv.ap(), 0, [[0, P], [1, C]]),
                    )
                    nc.sync.dma_start(
                        out=bcat[:, a * C : (a + 1) * C],
                        in_=_ap(bv.ap(), 0, [[0, P], [1, C]]),
                    )
                nc.scalar.activation(out=wb[:], in_=wcat[:], func=ACT.Copy,
                                     bias=0.0, scale=1.0)

            ident = pconst.tile([P, P], BF16, tag="ident")
            make_identity(nc, ident[:])

            # ones4: [ (h4,d):128, 4 ]  1 at row p, col p//32
            ones4 = pconst.tile([P, 4], BF16, tag="ones4")
            nc.gpsimd.memset(ones4[:], 0.0)
            for h4 in range(4):
                nc.gpsimd.memset(ones4[32 * h4 : 32 * (h4 + 1), h4 : h4 + 1], 1.0)

            # bd4: [ (s,j):128, 32 ]  1 at col j = p mod 32 (identity32 per s block)
            bd4 = pconst.tile([P, 32], BF16, tag="bd4")
            nc.gpsimd.memset(bd4[:], 0.0)
            for s in range(4):
                make_identity(nc, bd4[32 * s : 32 * (s + 1), :], nomemset=True)

            # ---------------- per tile ----------------
            state = {}

            def phase_load(i):
                t = i % NTILES
                qcat = pin.tile([P, FQ], F32, tag="qcat")
                qr = qcat[:]
                for xi in range(4):
                    for qh in range(2):
                        src = _ap(
                            in_aps[xi],
                            (8 * t + qh) * W * C,
                            [[2 * W * C, 4], [2 * C, NWI], [1, 2 * C]],
                        )
                        dst = _ap(
                            qr, xi * NTOK * C + qh * 2 * C, [qr.ap[0], [1, 2 * C]]
                        )
                        nc.sync.dma_start(out=dst, in_=src)
                qb = pqb.tile([P, FQ], BF16, tag="qb")
                nc.scalar.activation(out=qb[:, :2048], in_=qcat[:, :2048],
                                     func=ACT.Copy, bias=0.0, scale=1.0)
                nc.scalar.activation(out=qb[:, 2048:], in_=qcat[:, 2048:],
                                     func=ACT.Copy, bias=0.0, scale=1.0)
                state[i] = {"qb": qb}

            def phase_scores(i):
                """QT transposes (PE), products (DVE), S matmuls (PE), exp (ACT),
                softmax (DVE)."""
                st = state[i]
                qb = st["qb"]
                qbr = qb[:]

                # QT psum tiles; QT1/QT3 copied to SBUF (DVE may read only
                # one PSUM operand per instruction).
                qt = {}
                tag_of = {1: "qtA", 0: "qtB", 2: "qtC", 3: "qtA"}

                def do_qt(xi):
                    qt[xi] = zqt.tile([P, 1024], BF16, tag=tag_of[xi], name=f"qt{xi}")
                    for half in range(2):
                        for tk in range(4):
                            nc.tensor.transpose(
                                qt[xi][:, half * 512 + tk * 128 : half * 512 + tk * 128 + 128],
                                _ap(qbr, xi * NTOK * C + tk * C + half * 128,
                                    [qbr.ap[0], [1, 128]]),
                                ident[:],
                            )

                S = zs.tile([P, 512], F32, tag="S")

                def do_prod(q_ref, k_ref, eng=None):
                    pr = pprod.tile([P, 4096], BF16, tag="prod", name="prod")
                    (eng or nc.vector).tensor_tensor(
                        out=_ap(pr[:], 0, [pr[:].ap[0], [1, 4096]]),
                        in0=_ap(q_ref, 0,
                                [q_ref.ap[0], [512, 2], [128, 4], [0, 4], [1, 128]]),
                        in1=_ap(k_ref, 0,
                                [k_ref.ap[0], [512, 2], [0, 4], [128, 4], [1, 128]]),
                        op=ALU.mult,
                    )
                    xi = do_prod.xi
                    for half in range(2):
                        for tk in range(4):
                            for s in range(4):
                                nc.tensor.matmul(
                                    S[:, xi * 128 + tk * 32 + s * 8 + half * 4
                                      : xi * 128 + tk * 32 + s * 8 + half * 4 + 4],
                                    pr[:, half * 2048 + (tk * 4 + s) * 128
                                      : half * 2048 + (tk * 4 + s) * 128 + 128],
                                    ones4[:],
                                    start=True, stop=True,
                                )

                do_qt(1)
                qs1 = pqts.tile([P, 1024], BF16, tag="qs1")
                nc.scalar.activation(out=qs1[:], in_=qt[1][:], func=ACT.Copy,
                                     bias=0.0, scale=1.0)
                do_qt(0)
                do_prod.xi = 0
                do_prod(qt[0][:], qs1[:])
                do_qt(3)
                qs3 = pqts.tile([P, 1024], BF16, tag="qs3")
                nc.scalar.activation(out=qs3[:], in_=qt[3][:], func=ACT.Copy,
                                     bias=0.0, scale=1.0)
                do_qt(2)
                do_prod.xi = 1
                do_prod(qs1[:], qt[2][:])
                do_prod.xi = 2
                do_prod(qt[2][:], qs3[:])
                do_prod.xi = 3
                do_prod(qs3[:], qs1[:])


                e = psmall.tile([P, 512], BF16, tag="e", bufs=3)
                nc.scalar.activation(out=e[:], in_=S[:], func=ACT.Exp,
                                     bias=0.0, scale=INV_SQRT_D)
                er = e[:]
                z1 = psmall.tile([P, 256], BF16, tag="z1", bufs=1)
                nc.vector.tensor_tensor(
                    out=z1[:],
                    in0=_ap(er, 0, [er.ap[0], [32, 16], [8, 2], [1, 8]]),
                    in1=_ap(er, 16, [er.ap[0], [32, 16], [8, 2], [1, 8]]),
                    op=ALU.add,
                )
                z1r = z1[:]
                z = psmall.tile([P, 128], F32, tag="z", bufs=1)
                nc.vector.tensor_tensor(
                    out=z[:],
                    in0=_ap(z1r, 0, [z1r.ap[0], [16, 16], [1, 8]]),
                    in1=_ap(z1r, 8, [z1r.ap[0], [16, 16], [1, 8]]),
                    op=ALU.add,
                )
                rz = psmall.tile([P, 128], F32, tag="rz", bufs=1)
                nc.vector.reciprocal(out=rz[:], in_=z[:])
                rzr = rz[:]
                pn = psmall.tile([P, 512], BF16, tag="pn", bufs=3)
                nc.vector.tensor_tensor(
                    out=pn[:],
                    in0=e[:],
                    in1=_ap(rzr, 0, [rzr.ap[0], [8, 16], [0, 4], [1, 8]]),
                    op=ALU.mult,
                )
                st["pn"] = pn
                st["e"] = e

                kbs = {}
                for xi in range(4):
                    kbt = zkbt.tile([P, 1024], BF16, tag="kbt", name=f"kbt{xi}")
                    for d8 in range(8):
                        nc.tensor.transpose(
                            kbt[:, d8 * 128 : d8 * 128 + 128],
                            _ap(qbr, xi * NTOK * C + d8,
                                [qbr.ap[0], [8, 128]]),
                            ident[:],
                        )
                    kb_sb = pkbs.tile([P, 1024], BF16, tag=f"kbs{xi}")
                    nc.scalar.activation(out=kb_sb[:], in_=kbt[:], func=ACT.Copy,
                                         bias=0.0, scale=1.0)
                    kbs[xi] = kb_sb
                st["kbs"] = kbs

            def phase_av(i):
                """pnT/kbT transposes (PE), kbT sbuf copy (Pool), pvT products
                (DVE), s-reduce + residual matmuls (PE), convert (ACT)."""
                st = state[i]
                qbr = st["qb"][:]
                pnr = st["pn"][:]

                xsq = pxsq.tile([P, FQ], BF16, tag="xsq")

                kbs = st["kbs"]

                # pn replicated over d4 (Pool): free order (xi,t,s,h,d4)
                pn_rep = psmall.tile([P, 2048], BF16, tag="pn_rep", bufs=1)
                for xi_ in range(4):
                    nc.gpsimd.tensor_copy(
                        out=pn_rep[:, xi_ * 512 : xi_ * 512 + 512],
                        in_=_ap(pnr, xi_ * 128,
                                [pnr.ap[0], [32, 4], [8, 4], [1, 8], [0, 4]]),
                    )
                pn_rep_r = pn_rep[:]
                pnt_pair = {}
                for xi in range(4):
                    if xi % 2 == 0:
                        pnt_pair[xi // 2] = zpnt.tile(
                            [P, 1024], BF16, tag="pnt", name=f"pnt{xi // 2}")
                        for xj in (xi, xi + 1):
                            for tk in range(4):
                                nc.tensor.transpose(
                                    pnt_pair[xi // 2][:, (xj % 2) * 512 + tk * 128
                                                      : (xj % 2) * 512 + tk * 128 + 128],
                                    _ap(pn_rep_r, (xj * 4 + tk) * 128,
                                        [pn_rep_r.ap[0], [1, 128]]),
                                    ident[:],
                                )
                    pnt = pnt_pair[xi // 2][:, (xi % 2) * 512 : (xi % 2) * 512 + 512]
                    # AV products
                    pvt = ppvt.tile([P, FQ], BF16, tag="pvt", name=f"pvt{xi}")
                    pntr = pnt[:]
                    kbr = kbs[XK[xi]][:]
                    nc.vector.tensor_tensor(
                        out=_ap(pvt[:], 0, [pvt[:].ap[0], [1, FQ]]),
                        in0=_ap(pntr, 0, [pntr.ap[0], [128, 4], [0, 8], [1, 128]]),
                        in1=_ap(kbr, 0, [kbr.ap[0], [0, 4], [128, 8], [1, 128]]),
                        op=ALU.mult,
                    )
                    # s-reduce + residual matmuls into avout
                    av = zav.tile([P, 1024], F32, tag="av")
                    avr = av[:]
                    qb_sb = kbs[xi][:]
                    for tk in range(4):
                        for d8 in range(8):
                            o_ap = _ap(avr, tk * 256 + d8, [avr.ap[0], [8, 32]])
                            if AV_RESID:
                                nc.tensor.matmul(
                                    o_ap,
                                    _ap(qb_sb, d8 * 128, [qb_sb.ap[0], [1, 128]]),
                                    ident[:, 32 * tk : 32 * tk + 32],
                                    start=True, stop=not AV_ATT,
                                )
                            if AV_ATT:
                                nc.tensor.matmul(
                                    o_ap,
                                    pvt[:, (tk * 8 + d8) * 128 : (tk * 8 + d8) * 128 + 128],
                                    bd4[:],
                                    start=not AV_RESID, stop=True,
                                )
                    nc.scalar.activation(
                        out=xsq[:, xi * 1024 : xi * 1024 + 1024],
                        in_=av[:],
                        func=ACT.Copy, bias=0.0, scale=1.0,
                    )
                st["xsq"] = xsq

            def phase_ln(i):
                """sq-sums via ACT accumulator, x-tree (DVE), LN scalars,
                normalize, store."""
                st = state[i]
                xsq = st["xsq"]
                xr = xsq[:]
                stats = psmall.tile([P, 32], F32, tag="stats", bufs=1)
                sqscr = psmall.tile([P, 256], BF16, tag="sqscr", bufs=1)
                for k in range(16):
                    nc.scalar.activation(
                        out=sqscr[:],
                        in_=xsq[:, k * 256 : k * 256 + 256],
                        func=ACT.Square, bias=0.0, scale=1.0,
                        accum_out=stats[:, 16 + k : 17 + k],
                    )
                xtr = pxtr.tile([P, 2048], BF16, tag="xtr")
                xtrr = xtr[:]
                # L1: 256->128 per group (16 groups, x only)
                nc.vector.tensor_tensor(
                    out=_ap(xtrr, 0, [xtrr.ap[0], [128, 16], [1, 128]]),
                    in0=_ap(xr, 0, [xr.ap[0], [256, 16], [1, 128]]),
                    in1=_ap(xr, 128, [xr.ap[0], [256, 16], [1, 128]]),
                    op=ALU.add,
                )
                wdt = 64
                while wdt >= 2:
                    nc.vector.tensor_tensor(
                        out=_ap(xtrr, 0, [xtrr.ap[0], [128, 16], [1, wdt]]),
                        in0=_ap(xtrr, 0, [xtrr.ap[0], [128, 16], [1, wdt]]),
                        in1=_ap(xtrr, wdt, [xtrr.ap[0], [128, 16], [1, wdt]]),
                        op=ALU.add,
                    )
                    wdt //= 2
                nc.vector.tensor_tensor(
                    out=stats[:, :16],
                    in0=_ap(xtrr, 0, [xtrr.ap[0], [128, 16], [1, 1]]),
                    in1=_ap(xtrr, 1, [xtrr.ap[0], [128, 16], [1, 1]]),
                    op=ALU.add,
                )
                ms = psmall.tile([P, 32], F32, tag="ms")
                nc.scalar.activation(out=ms[:], in_=stats[:], func=ACT.Copy,
                                     bias=0.0, scale=1.0 / C)
                musq = psmall.tile([P, 16], F32, tag="musq", bufs=1)
                nc.vector.tensor_tensor(
                    out=musq[:], in0=ms[:, :16], in1=ms[:, :16], op=ALU.mult
                )
                vpe = psmall.tile([P, 16], F32, tag="vpe", bufs=1)
                nc.vector.tensor_tensor(
                    out=vpe[:], in0=ms[:, 16:], in1=musq[:], op=ALU.subtract
                )
                sqv = psmall.tile([P, 16], F32, tag="sqv", bufs=1)
                nc.scalar.activation(out=sqv[:], in_=vpe[:], func=ACT.Sqrt,
                                     bias=epst[:], scale=1.0)
                rs = psmall.tile([P, 16], F32, tag="rs")
                nc.vector.reciprocal(out=rs[:], in_=sqv[:])

                y1 = py.tile([P, FQ], BF16, tag="y1")
                for k in range(16):
                    nc.vector.tensor_scalar(
                        out=y1[:, k * 256 : k * 256 + 256],
                        in0=xsq[:, k * 256 : k * 256 + 256],
                        scalar1=ms[:, k : k + 1],
                        scalar2=rs[:, k % 16 : k % 16 + 1],
                        op0=ALU.subtract,
                        op1=ALU.mult,
                    )
                wbr = wb[:]
                nc.vector.tensor_tensor(
                    out=y1[:],
                    in0=y1[:],
                    in1=_ap(wbr, 0, [wbr.ap[0], [C, 4], [0, 4], [1, C]]),
                    op=ALU.mult,
                )
                y2 = y1
                st["y2"] = y2
                st["i"] = i

            def phase_ln_tail(i):
                st = state[i]
                y2 = st["y2"]
                xsq = st["xsq"]
                y1 = y2
                stats = None
                y = py.tile([P, FQ], F32, tag="y")
                if DEBUG_DUMP is not None:
                    nc.vector.memset(y[:], 0.0)
                    dmp = {"e": st.get("e"), "pn": st.get("pn"),
                           "xres": xsq}[DEBUG_DUMP]
                    w_ = min(dmp.shape[1], FQ)
                    nc.scalar.activation(out=y[:, :w_], in_=dmp[:][:, :w_],
                                         func=ACT.Copy, bias=0.0, scale=1.0)
                yr = y[:]
                br = bcat[:]
                if DEBUG_DUMP is None:
                    eng_b = nc.vector if st["i"] == NT - 1 else nc.gpsimd
                    eng_b.tensor_tensor(
                        out=_ap(yr, 0, [yr.ap[0], [C, 4], [NX * C, 4], [1, C]]),
                        in0=y2[:],
                        in1=_ap(br, 0, [br.ap[0], [C, 4], [0, 4], [1, C]]),
                        op=ALU.add,
                    )
                t = i % NTILES
                for qh in range(2):
                    dst = _ap(
                        out_ap,
                        (8 * t + qh) * W * NX * C,
                        [[2 * W * NX * C, 4], [2 * NX * C, NWI], [1, 2 * NX * C]],
                    )
                    src = _ap(yr, qh * 2 * NX * C, [yr.ap[0], [1, 2 * NX * C]])
                    nc.sync.dma_start(out=dst, in_=src)
                del state[i]

            # ---- software pipeline: stagger by one tile
            phase_load(0)
            for i in range(NT):
                if i + 1 < NT:
                    phase_load(i + 1)
                phase_scores(i)
                if i == 0:
                    load_consts()
                phase_av(i)
                if i >= 1:
                    phase_ln(i - 1)
                if i >= 2:
                    phase_ln_tail(i - 2)
            phase_ln(NT - 1)
            phase_ln_tail(NT - 2)
            phase_ln_tail(NT - 1)
    return nc


def _split_multi_waits(nc):
    wid = 0
    for fn in nc.m.functions:
        for blk in fn.blocks:
            new_list = []
            changed = False
            for inst in blk.instructions:
                si = inst.sync_info
                if si is not None:
                    waits = list(si.on_wait)
                    if len(waits) > 1:
                        for w in waits[:-1]:
                            ev = mybir.InstEventSemaphore(
                                name=f"WSPLIT-{wid}", ins=[], outs=[]
                            )
                            wid += 1
                            ev.engine = inst.engine
                            ev.sync_info = bass_rust.SyncInfo(on_wait=[w], on_update=[])
                            new_list.append(ev)
                        inst.sync_info = bass_rust.SyncInfo(
                            on_wait=[waits[-1]], on_update=list(si.on_update)
                        )
                        changed = True
                new_list.append(inst)
            if changed:
                blk.instructions = new_list


_NC_CACHE = None


def _get_nc():
    global _NC_CACHE
    if _NC_CACHE is None:
        nc = build_kernel()
        _split_multi_waits(nc)
        _NC_CACHE = nc
    return _NC_CACHE


def kernel(**inputs) -> np.ndarray:
    nc = _get_nc()
    param_names = [f"ln{a + 1}_{s}" for a in range(4) for s in ("w", "b")]
    in_maps = []
    for ci in range(B):
        m = {
            name: np.ascontiguousarray(np.asarray(inputs[name])[ci], dtype=np.float32)
            for name in ("r", "g", "b", "ir")
        }
        for pnm in param_names:
            m[pnm] = np.ascontiguousarray(np.asarray(inputs[pnm]), dtype=np.float32)
        in_maps.append(m)
    try:
        res = run_bass_kernel_spmd(nc, in_maps, list(range(B)))
    except ModuleNotFoundError:
        import os

        os.environ["BASS_NEVER_TRACE"] = "1"
        res = run_bass_kernel_spmd(nc, in_maps, list(range(B)))
    return np.stack([res.results[ci]["out"] for ci in range(B)], axis=0)


if __name__ == "__main__":
    from concourse.timeline_sim import TimelineSim

    nc = build_kernel()
    _split_multi_waits(nc)
    t = TimelineSim(nc).simulate()
    print(f"TimelineSim: {t:.0f} ns")


# revision 10
# speedup vs baseline: 1.1586x; 1.0034x over previous
"""Trainium2 Bass kernel for CAttentionBlock — v6: PE-offloaded reductions.

Layout per tile of 128 windows (all 4 attentions):
  qb   [128w, (xi,tok,c)=4096] bf16, natural channel order c=h*32+d
  QT   (PE transpose) [c128, (half,t,w)] bf16 psum, per xi
  prod (DVE) [c128, (t,s,w)=2048] bf16 sbuf, per (xi,half)
  S    (PE mm: lhsT=prod chunk, rhs=ones4 blockdiag) [128w, (xi,t,s,h8)=512] f32 psum
  e/pn (ACT exp + DVE softmax) [128w, 512] bf16
  pnT  (PE transpose w/ stride-0 d4 replication) [(s,h,d4)128, (t,w)] bf16 psum, per xi
  kbT  (PE transpose) [(tok,h,d4)128, (d8,w)] bf16 psum -> sbuf copy (Pool), per xi
  pvT  (DVE) [(s,h,d4)128, (t,d8,w)=4096] bf16 sbuf, per xi
  avout(PE mm pairs: s-reduce via bd4 + residual via identity slice)
       [128w, (t,c)=1024] f32 psum, per xi
  xres (ACT copy) -> xsq [128w, (xi,t,c)] bf16 sbuf; sq (ACT Square)
  stats (DVE halving tree) -> mu/rs (ACT+DVE tiny)
  norm: 16x tensor_scalar (x-mu)*rs (4x mode), *w (DVE TT), +b -> f32 (Pool)
"""

import sys

for _p in ("/opt/trn_rl_repo",):
    if _p not in sys.path:
        sys.path.insert(0, _p)

import numpy as np

import bass_rust
import concourse.bass as bass
import concourse.tile as tile
from concourse import mybir
from concourse.bass_utils import run_bass_kernel_spmd
from concourse.masks import make_identity

F32 = mybir.dt.float32
BF16 = mybir.dt.bfloat16
ALU = mybir.AluOpType
ACT = mybir.ActivationFunctionType
AX = mybir.AxisListType

B, H, W, C = 8, 64, 64, 256
WS = 2
NH = 8
D = C // NH            # 32
NTOK = WS * WS         # 4
NW = (H // WS) * (W // WS)
NWI = H // WS          # 32
P = 128
NTILES = NW // P       # 8
NX = 4
EPS = 1e-5
INV_SQRT_D = 1.0 / float(np.sqrt(D))
XK = [1, 2, 3, 1]      # K/V input index per attention

FQ = NX * NTOK * C         # 4096
DEBUG_DUMP = None  # "e" | "pn" | "xres" | "stats" | "y1"
AV_RESID = True
AV_ATT = True


def _ap(ref, offset_delta, dims):
    return bass_rust.AP(ref.tensor, ref.offset + offset_delta, [list(d) for d in dims])


def build_kernel(reps=1):
    nc = bass.Bass("TRN2", target_bir_lowering=False, debug=False)

    ins = {
        name: nc.dram_tensor(name, [H, W, C], F32, kind="ExternalInput")
        for name in ("r", "g", "b", "ir")
    }
    ln_params = []
    for a in range(4):
        wv = nc.dram_tensor(f"ln{a + 1}_w", [C], F32, kind="ExternalInput")
        bv = nc.dram_tensor(f"ln{a + 1}_b", [C], F32, kind="ExternalInput")
        ln_params.append((wv, bv))
    out = nc.dram_tensor("out", [H, W, 4 * C], F32, kind="ExternalOutput")

    in_aps = [ins[n].ap() for n in ("r", "g", "b", "ir")]
    out_ap = out.ap()
    NT = NTILES * reps

    with tile.TileContext(nc) as tc:
        with (
            tc.tile_pool(name="const", bufs=1) as pconst,
            tc.tile_pool(name="pin", bufs=2) as pin,
            tc.tile_pool(name="pqb", bufs=2) as pqb,
            tc.tile_pool(name="pprod", bufs=2) as pprod,
            tc.tile_pool(name="pqts", bufs=2) as pqts,
            tc.tile_pool(name="ppvt", bufs=3) as ppvt,
            tc.tile_pool(name="pkbs", bufs=1) as pkbs,
            tc.tile_pool(name="pxsq", bufs=2) as pxsq,
            tc.tile_pool(name="pxtr", bufs=1) as pxtr,
            tc.tile_pool(name="py", bufs=2) as py,
            tc.tile_pool(name="psmall", bufs=2) as psmall,
            tc.tile_pool(name="zqt", bufs=1, space="PSUM") as zqt,
            tc.tile_pool(name="zpnt", bufs=1, space="PSUM") as zpnt,
            tc.tile_pool(name="zkbt", bufs=1, space="PSUM") as zkbt,
            tc.tile_pool(name="zs", bufs=1, space="PSUM") as zs,
            tc.tile_pool(name="zav", bufs=1, space="PSUM") as zav,
        ):
            # ---------------- constants (DMAs deferred past tile-0 loads) ----
            wcat = pconst.tile([P, NX * C], F32, tag="wcat")
            bcat = pconst.tile([P, NX * C], F32, tag="bcat")
            epst = pconst.tile([P, 1], F32, tag="epst")
            nc.vector.memset(epst[:], EPS)
            wb = pconst.tile([P, NX * C], BF16, tag="wb")

            def load_consts():
                for a, (wv, bv) in enumerate(ln_params):
                    nc.sync.dma_start(
                        out=wcat[:, a * C : (a + 1) * C],
                        in_=_ap(wv.ap(), 0, [[0, P], [1, C]]),
                    )
                    nc.sync.dma_start(
                        out=bcat[:, a * C : (a + 1) * C],
                        in_=_ap(bv.ap(), 0, [[0, P], [1, C]]),
                    )
                nc.scalar.activation(out=wb[:], in_=wcat[:], func=ACT.Copy,
                                     bias=0.0, scale=1.0)

            ident = pconst.tile([P, P], BF16, tag="ident")
            make_identity(nc, ident[:])

            # ones4: [ (h4,d):128, 4 ]  1 at row p, col p//32
            ones4 = pconst.tile([P, 4], BF16, tag="ones4")
            nc.gpsimd.memset(ones4[:], 0.0)
            for h4 in range(4):
                nc.gpsimd.memset(ones4[32 * h4 : 32 * (h4 + 1), h4 : h4 + 1], 1.0)

            # bd4: [ (s,j):128, 32 ]  1 at col j = p mod 32 (identity32 per s block)
            bd4 = pconst.tile([P, 32], BF16, tag="bd4")
            nc.gpsimd.memset(bd4[:], 0.0)
            for s in range(4):
                make_identity(nc, bd4[32 * s : 32 * (s + 1), :], nomemset=True)

            # ---------------- per tile ----------------
            state = {}

            def phase_load(i):
                t = i % NTILES
                qcat = pin.tile([P, FQ], F32, tag="qcat")
                qr = qcat[:]
                for xi in range(4):
                    for qh in range(2):
                        src = _ap(
                            in_aps[xi],
                            (8 * t + qh) * W * C,
                            [[2 * W * C, 4], [2 * C, NWI], [1, 2 * C]],
                        )
                        dst = _ap(
                            qr, xi * NTOK * C + qh * 2 * C, [qr.ap[0], [1, 2 * C]]
                        )
                        nc.sync.dma_start(out=dst, in_=src)
                qb = pqb.tile([P, FQ], BF16, tag="qb")
                nc.scalar.activation(out=qb[:, :2048], in_=qcat[:, :2048],
                                     func=ACT.Copy, bias=0.0, scale=1.0)
                nc.scalar.activation(out=qb[:, 2048:], in_=qcat[:, 2048:],
                                     func=ACT.Copy, bias=0.0, scale=1.0)
                state[i] = {"qb": qb}

            def phase_scores(i):
                """QT transposes (PE), products (DVE), S matmuls (PE), exp (ACT),
                softmax (DVE)."""
                st = state[i]
                qb = st["qb"]
                qbr = qb[:]

                # QT psum tiles; QT1/QT3 copied to SBUF (DVE may read only
                # one PSUM operand per instruction).
                qt = {}
                tag_of = {1: "qtA", 0: "qtB", 2: "qtC", 3: "qtA"}

                def do_qt(xi):
                    qt[xi] = zqt.tile([P, 1024], BF16, tag=tag_of[xi], name=f"qt{xi}")
                    for half in range(2):
                        for tk in range(4):
                            nc.tensor.transpose(
                                qt[xi][:, half * 512 + tk * 128 : half * 512 + tk * 128 + 128],
                                _ap(qbr, xi * NTOK * C + tk * C + half * 128,
                                    [qbr.ap[0], [1, 128]]),
                                ident[:],
                            )

                S = zs.tile([P, 512], F32, tag="S")

                def do_prod(q_ref, k_ref, eng=None):
                    pr = pprod.tile([P, 4096], BF16, tag="prod", name="prod")
                    (eng or nc.vector).tensor_tensor(
                        out=_ap(pr[:], 0, [pr[:].ap[0], [1, 4096]]),
                        in0=_ap(q_ref, 0,
                                [q_ref.ap[0], [512, 2], [128, 4], [0, 4], [1, 128]]),
                        in1=_ap(k_ref, 0,
                                [k_ref.ap[0], [512, 2], [0, 4], [128, 4], [1, 128]]),
                        op=ALU.mult,
                    )
                    xi = do_prod.xi
                    for half in range(2):
                        for tk in range(4):
                            for s in range(4):
                                nc.tensor.matmul(
                                    S[:, xi * 128 + tk * 32 + s * 8 + half * 4
                                      : xi * 128 + tk * 32 + s * 8 + half * 4 + 4],
                                    pr[:, half * 2048 + (tk * 4 + s) * 128
                                      : half * 2048 + (tk * 4 + s) * 128 + 128],
                                    ones4[:],
                                    start=True, stop=True,
                                )

                do_qt(1)
                qs1 = pqts.tile([P, 1024], BF16, tag="qs1")
                nc.scalar.activation(out=qs1[:], in_=qt[1][:], func=ACT.Copy,
                                     bias=0.0, scale=1.0)
                do_qt(0)
                do_prod.xi = 0
                do_prod(qt[0][:], qs1[:])
                do_qt(3)
                qs3 = pqts.tile([P, 1024], BF16, tag="qs3")
                nc.scalar.activation(out=qs3[:], in_=qt[3][:], func=ACT.Copy,
                                     bias=0.0, scale=1.0)
                do_qt(2)
                do_prod.xi = 1
                do_prod(qs1[:], qt[2][:])
                do_prod.xi = 2
                do_prod(qt[2][:], qs3[:])
                do_prod.xi = 3
                do_prod(qs3[:], qs1[:])


                e = psmall.tile([P, 512], BF16, tag="e", bufs=3)
                nc.scalar.activation(out=e[:], in_=S[:], func=ACT.Exp,
                                     bias=0.0, scale=INV_SQRT_D)
                er = e[:]
                z1 = psmall.tile([P, 256], BF16, tag="z1", bufs=1)
                nc.vector.tensor_tensor(
                    out=z1[:],
                    in0=_ap(er, 0, [er.ap[0], [32, 16], [8, 2], [1, 8]]),
                    in1=_ap(er, 16, [er.ap[0], [32, 16], [8, 2], [1, 8]]),
                    op=ALU.add,
                )
                z1r = z1[:]
                z = psmall.tile([P, 128], F32, tag="z", bufs=1)
                nc.vector.tensor_tensor(
                    out=z[:],
                    in0=_ap(z1r, 0, [z1r.ap[0], [16, 16], [1, 8]]),
                    in1=_ap(z1r, 8, [z1r.ap[0], [16, 16], [1, 8]]),
                    op=ALU.add,
                )
                rz = psmall.tile([P, 128], F32, tag="rz", bufs=1)
                nc.vector.reciprocal(out=rz[:], in_=z[:])
                rzr = rz[:]
                pn = psmall.tile([P, 512], BF16, tag="pn", bufs=3)
                nc.vector.tensor_tensor(
                    out=pn[:],
                    in0=e[:],
                    in1=_ap(rzr, 0, [rzr.ap[0], [8, 16], [0, 4], [1, 8]]),
                    op=ALU.mult,
                )
                st["pn"] = pn
                st["e"] = e

                kbs = {}
                for xi in range(4):
                    kbt = zkbt.tile([P, 1024], BF16, tag="kbt", name=f"kbt{xi}")
                    for d8 in range(8):
                        nc.tensor.transpose(
                            kbt[:, d8 * 128 : d8 * 128 + 128],
                            _ap(qbr, xi * NTOK * C + d8,
                                [qbr.ap[0], [8, 128]]),
                            ident[:],
                        )
                    kb_sb = pkbs.tile([P, 1024], BF16, tag=f"kbs{xi}")
                    nc.scalar.activation(out=kb_sb[:], in_=kbt[:], func=ACT.Copy,
                                         bias=0.0, scale=1.0)
                    kbs[xi] = kb_sb
                st["kbs"] = kbs

            def phase_av(i):
                """pnT/kbT transposes (PE), kbT sbuf copy (Pool), pvT products
                (DVE), s-reduce + residual matmuls (PE), convert (ACT)."""
                st = state[i]
                qbr = st["qb"][:]
                pnr = st["pn"][:]

                xsq = pxsq.tile([P, FQ], BF16, tag="xsq")

                kbs = st["kbs"]

                # pn replicated over d4 (Pool): free order (xi,t,s,h,d4)
                pn_rep = psmall.tile([P, 2048], BF16, tag="pn_rep", bufs=1)
                for xi_ in range(4):
                    nc.gpsimd.tensor_copy(
                        out=pn_rep[:, xi_ * 512 : xi_ * 512 + 512],
                        in_=_ap(pnr, xi_ * 128,
                                [pnr.ap[0], [32, 4], [8, 4], [1, 8], [0, 4]]),
                    )
                pn_rep_r = pn_rep[:]
                pnt_pair = {}
                for xi in range(4):
                    if xi % 2 == 0:
                        pnt_pair[xi // 2] = zpnt.tile(
                            [P, 1024], BF16, tag="pnt", name=f"pnt{xi // 2}")
                        for xj in (xi, xi + 1):
                            for tk in range(4):
                                nc.tensor.transpose(
                                    pnt_pair[xi // 2][:, (xj % 2) * 512 + tk * 128
                                                      : (xj % 2) * 512 + tk * 128 + 128],
                                    _ap(pn_rep_r, (xj * 4 + tk) * 128,
                                        [pn_rep_r.ap[0], [1, 128]]),
                                    ident[:],
                                )
                    pnt = pnt_pair[xi // 2][:, (xi % 2) * 512 : (xi % 2) * 512 + 512]
                    # AV products
                    pvt = ppvt.tile([P, FQ], BF16, tag="pvt", name=f"pvt{xi}")
                    pntr = pnt[:]
                    kbr = kbs[XK[xi]][:]
                    nc.vector.tensor_tensor(
                        out=_ap(pvt[:], 0, [pvt[:].ap[0], [1, FQ]]),
                        in0=_ap(pntr, 0, [pntr.ap[0], [128, 4], [0, 8], [1, 128]]),
                        in1=_ap(kbr, 0, [kbr.ap[0], [0, 4], [128, 8], [1, 128]]),
                        op=ALU.mult,
                    )
                    # s-reduce + residual matmuls into avout
                    av = zav.tile([P, 1024], F32, tag="av")
                    avr = av[:]
                    qb_sb = kbs[xi][:]
                    for tk in range(4):
                        for d8 in range(8):
                            o_ap = _ap(avr, tk * 256 + d8, [avr.ap[0], [8, 32]])
                            if AV_RESID:
                                nc.tensor.matmul(
                                    o_ap,
                                    _ap(qb_sb, d8 * 128, [qb_sb.ap[0], [1, 128]]),
                                    ident[:, 32 * tk : 32 * tk + 32],
                                    start=True, stop=not AV_ATT,
                                )
                            if AV_ATT:
                                nc.tensor.matmul(
                                    o_ap,
                                    pvt[:, (tk * 8 + d8) * 128 : (tk * 8 + d8) * 128 + 128],
                                    bd4[:],
                                    start=not AV_RESID, stop=True,
                                )
                    nc.scalar.activation(
                        out=xsq[:, xi * 1024 : xi * 1024 + 1024],
                        in_=av[:],
                        func=ACT.Copy, bias=0.0, scale=1.0,
                    )
                st["xsq"] = xsq

            def phase_ln(i):
                """sq-sums via ACT accumulator, x-tree (DVE), LN scalars,
                normalize, store."""
                st = state[i]
                xsq = st["xsq"]
                xr = xsq[:]
                stats = psmall.tile([P, 32], F32, tag="stats", bufs=1)
                sqscr = psmall.tile([P, 256], BF16, tag="sqscr", bufs=1)
                for k in range(16):
                    nc.scalar.activation(
                        out=sqscr[:],
                        in_=xsq[:, k * 256 : k * 256 + 256],
                        func=ACT.Square, bias=0.0, scale=1.0,
                        accum_out=stats[:, 16 + k : 17 + k],
                    )
                xtr = pxtr.tile([P, 2048], BF16, tag="xtr")
                xtrr = xtr[:]
                # L1: 256->128 per group (16 groups, x only)
                nc.vector.tensor_tensor(
                    out=_ap(xtrr, 0, [xtrr.ap[0], [128, 16], [1, 128]]),
                    in0=_ap(xr, 0, [xr.ap[0], [256, 16], [1, 128]]),
                    in1=_ap(xr, 128, [xr.ap[0], [256, 16], [1, 128]]),
                    op=ALU.add,
                )
                wdt = 64
                while wdt >= 2:
                    nc.vector.tensor_tensor(
                        out=_ap(xtrr, 0, [xtrr.ap[0], [128, 16], [1, wdt]]),
                        in0=_ap(xtrr, 0, [xtrr.ap[0], [128, 16], [1, wdt]]),
                        in1=_ap(xtrr, wdt, [xtrr.ap[0], [128, 16], [1, wdt]]),
                        op=ALU.add,
                    )
                    wdt //= 2
                nc.vector.tensor_tensor(
                    out=stats[:, :16],
                    in0=_ap(xtrr, 0, [xtrr.ap[0], [128, 16], [1, 1]]),
                    in1=_ap(xtrr, 1, [xtrr.ap[0], [128, 16], [1, 1]]),
                    op=ALU.add,
                )
                ms = psmall.tile([P, 32], F32, tag="ms")
                nc.scalar.activation(out=ms[:], in_=stats[:], func=ACT.Copy,
                                     bias=0.0, scale=1.0 / C)
                musq = psmall.tile([P, 16], F32, tag="musq", bufs=1)
                nc.vector.tensor_tensor(
                    out=musq[:], in0=ms[:, :16], in1=ms[:, :16], op=ALU.mult
                )
                vpe = psmall.tile([P, 16], F32, tag="vpe", bufs=1)
                nc.vector.tensor_tensor(
                    out=vpe[:], in0=ms[:, 16:], in1=musq[:], op=ALU.subtract
                )
                sqv = psmall.tile([P, 16], F32, tag="sqv", bufs=1)
                nc.scalar.activation(out=sqv[:], in_=vpe[:], func=ACT.Sqrt,
                                     bias=epst[:], scale=1.0)
                rs = psmall.tile([P, 16], F32, tag="rs")
                nc.vector.reciprocal(out=rs[:], in_=sqv[:])

                y1 = py.tile([P, FQ], BF16, tag="y1")
                for k in range(16):
                    nc.vector.tensor_scalar(
                        out=y1[:, k * 256 : k * 256 + 256],
                        in0=xsq[:, k * 256 : k * 256 + 256],
                        scalar1=ms[:, k : k + 1],
                        scalar2=rs[:, k % 16 : k % 16 + 1],
                        op0=ALU.subtract,
                        op1=ALU.mult,
                    )
                wbr = wb[:]
                nc.vector.tensor_tensor(
                    out=y1[:],
                    in0=y1[:],
                    in1=_ap(wbr, 0, [wbr.ap[0], [C, 4], [0, 4], [1, C]]),
                    op=ALU.mult,
                )
                y2 = y1
                st["y2"] = y2
                st["i"] = i

            def phase_ln_tail(i):
                st = state[i]
                y2 = st["y2"]
                xsq = st["xsq"]
                y1 = y2
                stats = None
                y = py.tile([P, FQ], F32, tag="y")
                if DEBUG_DUMP is not None:
                    nc.vector.memset(y[:], 0.0)
                    dmp = {"e": st.get("e"), "pn": st.get("pn"),
                           "xres": xsq}[DEBUG_DUMP]
                    w_ = min(dmp.shape[1], FQ)
                    nc.scalar.activation(out=y[:, :w_], in_=dmp[:][:, :w_],
                                         func=ACT.Copy, bias=0.0, scale=1.0)
                yr = y[:]
                br = bcat[:]
                if DEBUG_DUMP is None:
                    eng_b = nc.vector if st["i"] == NT - 1 else nc.gpsimd
                    eng_b.tensor_tensor(
                        out=_ap(yr, 0, [yr.ap[0], [C, 4], [NX * C, 4], [1, C]]),
                        in0=y2[:],
                        in1=_ap(br, 0, [br.ap[0], [C, 4], [0, 4], [1, C]]),
                        op=ALU.add,
                    )
                t = i % NTILES
                for qh in range(2):
                    dst = _ap(
                        out_ap,
                        (8 * t + qh) * W * NX * C,
                        [[2 * W * NX * C, 4], [2 * NX * C, NWI], [1, 2 * NX * C]],
                    )
                    src = _ap(yr, qh * 2 * NX * C, [yr.ap[0], [1, 2 * NX * C]])
                    nc.sync.dma_start(out=dst, in_=src)
                del state[i]

            # ---- software pipeline: stagger by one tile
            phase_load(0)
            for i in range(NT):
                if i + 1 < NT:
                    phase_load(i + 1)
                phase_scores(i)
                if i == 0:
                    load_consts()
                phase_av(i)
                if i >= 1:
                    phase_ln(i - 1)
                if i >= 2:
                    phase_ln_tail(i - 2)
            phase_ln(NT - 1)
            phase_ln_tail(NT - 2)
            phase_ln_tail(NT - 1)
    return nc


def _split_multi_waits(nc):
    wid = 0
    for fn in nc.m.functions:
        for blk in fn.blocks:
            new_list = []
            changed = False
            for inst in blk.instructions:
                si = inst.sync_info
                if si is not None:
                    waits = list(si.on_wait)
                    if len(waits) > 1:
                        for w in waits[:-1]:
                            ev = mybir.InstEventSemaphore(
                                name=f"WSPLIT-{wid}", ins=[], outs=[]
                            )
                            wid += 1
                            ev.engine = inst.engine
                            ev.sync_info = bass_rust.SyncInfo(on_wait=[w], on_update=[])
                            new_list.append(ev)
                        inst.sync_info = bass_rust.SyncInfo(
                            on_wait=[waits[-1]], on_update=list(si.on_update)
                        )
                        changed = True
                new_list.append(inst)
            if changed:
                blk.instructions = new_list


_NC_CACHE = None


def _get_nc():
    global _NC_CACHE
    if _NC_CACHE is None:
        nc = build_kernel()
        _split_multi_waits(nc)
        _NC_CACHE = nc
    return _NC_CACHE


def kernel(**inputs) -> np.ndarray:
    nc = _get_nc()
    param_names = [f"ln{a + 1}_{s}" for a in range(4) for s in ("w", "b")]
    in_maps = []
    for ci in range(B):
        m = {
            name: np.ascontiguousarray(np.asarray(inputs[name])[ci], dtype=np.float32)
            for name in ("r", "g", "b", "ir")
        }
        for pnm in param_names:
            m[pnm] = np.ascontiguousarray(np.asarray(inputs[pnm]), dtype=np.float32)
        in_maps.append(m)
    try:
        res = run_bass_kernel_spmd(nc, in_maps, list(range(B)))
    except ModuleNotFoundError:
        import os

        os.environ["BASS_NEVER_TRACE"] = "1"
        res = run_bass_kernel_spmd(nc, in_maps, list(range(B)))
    return np.stack([res.results[ci]["out"] for ci in range(B)], axis=0)


if __name__ == "__main__":
    from concourse.timeline_sim import TimelineSim

    nc = build_kernel()
    _split_multi_waits(nc)
    t = TimelineSim(nc).simulate()
    print(f"TimelineSim: {t:.0f} ns")
